# revision 15
# baseline (speedup 1.0000x reference)
"""BitLinear (BitNet b1.58) Trainium2 kernel, 8-core data-parallel.

Per core (4096 tokens sharded over batch*seq, weight replicated and fed
host-transposed as wT = W.T, a layout/sharding choice):
    q  = round(x*scale)  integers in [-127,127]   -> exact in fp16/bf16
    t  = clip(round(W/s),-1,1) in {-1,0,1}
    out = (q @ t.T) * (absmax*s/127) per token, stored bf16 (host->fp32).

W-quant is 2 passes: DVE y = w*(1/s) + 12582912 (the fp32 magic add
IS the single-rounding RNE to integer - any smaller bias pre-rounds
the fraction at ulp(bias) and flips boundary weights, each of which
corrupts a whole output column), then ACT t = Sign(y - 12582912):
since |round(w/s)| <= 2, Sign IS the clip to {-1,0,1}.  (A biased
bf16(w/s + 192) variant was tried and rejected: double rounding gave
rel err 2.3e-2.  activation accum_out measures PRE-cast values, so it
cannot supply sum(q) either.)

Engine plan per 128-token tile (steady state, PE-bound ~3.9us):
    SP   : x DMA in, out DMA (both on the sync ring - keeps ACT clean)
    DVE  : absmax reduce, 1/absmax, scl, coef, oh1 output scale,
           [512:1024] drain half
    ACT  : z16 = fp16(x*scl + 1536), [0:512] drain half, oh0 out scale
    PE   : 8 fp16 transposes + 16 bf16 matmuls (fp32 PSUM exact)

Weight phase: rings balanced at 2.5MB each (sync: x0,w0,w2,w4,w6,x2,x4;
scalar: x1,w1,w3,w5,w7,x3,x5) so x0/x1 land first and tiles 0/1 are
fully quantized+transposed BEFORE s is ready; first matmul is then
bound only by the first ternary chunk.  Per-chunk-gated PE warmup
transposes keep the clock at max through the whole stream (PE drops to
0.65/1.2 GHz p-states when idle).  ACT table preloaded with a dummy op
(one load covers all afns).  Last |w| colsum split ACT/DVE.  GPSIMD is
never used for bulk elementwise work (measured 14.7us per 1024-elem
op).  DMA-XBAR transpose rejected: ~1200-descriptor storm per tile.
"""

import numpy as np

import concourse.bass as bass
import concourse.mybir as mybir
from concourse import tile, masks
from concourse.bass_utils import run_bass_kernel_spmd

F32 = mybir.dt.float32
BF16 = mybir.dt.bfloat16
FP16 = mybir.dt.float16

N_CORES = 8
B, S, D_IN, D_OUT = 4, 8192, 1024, 1024
TOKENS = B * S                     # 32768
TOK_PER_CORE = TOKENS // N_CORES   # 4096
TILES = TOK_PER_CORE // 128        # 32
KT = D_IN // 128                   # 8 contraction k-chunks

QMAX = 127.0
MAGIC16 = 1536.0     # z16 = fp16(x*scl + 1536): fp16 ulp=1 on [1024,2048)
MAGIC = 12582912.0   # 1.5 * 2**23: fp32 ulp=1 -> the add rounds RNE
# engine balance for W-quant: chunks whose round runs on gpsimd, and whose
# clip runs as a 3-op tensor_scalar chain (on gpsimd/DVE) instead of ACT Sign
GPS_ROUND_KS = (1, 5)
GPS3OP_KS = (5,)
DVE3OP_KS = (3,)


def _split_multiwaits(nc):
    """walrus here encodes at most ONE sem wait per instruction; Tile's tail
    drain (and occasionally other insts) carry several.  Split extras into
    single-wait NOPs on the same engine, preserving order."""
    for f in nc.m.functions:
        for bb in f.blocks:
            insts = list(bb.instructions)
            if not any(
                i.sync_info and len(i.sync_info.on_wait) > 1 for i in insts
            ):
                continue
            new = []
            for ins in insts:
                si = ins.sync_info
                if si and len(si.on_wait) > 1:
                    waits = list(si.on_wait)
                    for j, w in enumerate(waits[:-1]):
                        nop = mybir.InstNoOp(
                            name=f"{ins.name}_wsp{j}", ins=[], outs=[]
                        )
                        nop.engine = ins.engine
                        nop.sync_info = mybir.SyncInfo(on_wait=[w], on_update=[])
                        new.append(nop)
                    ins.sync_info = mybir.SyncInfo(
                        on_wait=[waits[-1]], on_update=list(si.on_update)
                    )
                new.append(ins)
            bb.instructions = new


def build_program():
    nc = bass.Bass(trn_type="TRN2")
    x_d = nc.dram_tensor("x", [TOK_PER_CORE, D_IN], F32, kind="ExternalInput")
    w_d = nc.dram_tensor("wT", [D_IN, D_OUT], F32, kind="ExternalInput")
    o_d = nc.dram_tensor("out", [TOK_PER_CORE, D_OUT], BF16, kind="ExternalOutput")

    Copy = mybir.ActivationFunctionType.Copy
    Sign = mybir.ActivationFunctionType.Sign
    Abs = mybir.ActivationFunctionType.Abs
    AX = mybir.AxisListType.X
    op = mybir.AluOpType

    with tile.TileContext(nc) as tc:
        from contextlib import ExitStack

        with ExitStack() as ctx:
            singles = ctx.enter_context(tc.tile_pool(name="singles", bufs=1))

            ident = singles.tile([128, 128], FP16)
            masks.make_identity(nc, ident[:])
            ident_f32 = singles.tile([128, 128], F32)
            masks.make_identity(nc, ident_f32[:])
            ident_bf = singles.tile([128, 128], BF16)
            masks.make_identity(nc, ident_bf[:])
            ones_col = singles.tile([128, 1], F32)
            nc.vector.memset(ones_col[:], 1.0)
            ones_row = singles.tile([1, 128], F32)
            nc.vector.memset(ones_row[:], 1.0)
            bc2 = singles.tile([128, 2], F32)    # [s, 1/s] broadcast to 128 parts
            scoef = singles.tile([128, 1], F32)  # s/127 broadcast
            negm = singles.tile([128, 1], F32)   # -MAGIC bias for the Sign pass
            nc.vector.memset(negm[:], -MAGIC)
            preheat = singles.tile([128, 1], F32)

            tT = [singles.tile([128, D_OUT], BF16, name=f"tT{k}", tag=f"tT{k}") for k in range(KT)]

            xpool = ctx.enter_context(tc.tile_pool(name="xpool", bufs=8))
            xmpool = ctx.enter_context(tc.tile_pool(name="xmpool", bufs=3))
            qtpool = ctx.enter_context(tc.tile_pool(name="qtpool", bufs=6))
            outpool = ctx.enter_context(tc.tile_pool(name="outpool", bufs=3))
            smpool = ctx.enter_context(tc.tile_pool(name="smpool", bufs=18))
            psq = ctx.enter_context(tc.tile_pool(name="psq", bufs=2, space="PSUM"))
            pso = ctx.enter_context(tc.tile_pool(name="pso", bufs=5, space="PSUM"))
            psw = ctx.enter_context(tc.tile_pool(name="psw", bufs=1, space="PSUM"))

            live = {}

            def a_dma_issue(n, eng=None):
                """x tile DMA issue only."""
                x_t = xpool.tile([128, D_IN], F32, tag="x")
                (eng or nc.sync).dma_start(x_t[:], x_d[n * 128:(n + 1) * 128, :])
                live[("x", n)] = x_t

            def a_stats(n):
                """per-token absmax/scale smalls (DVE)."""
                x_t = live[("x", n)]
                am = smpool.tile([128, 1], F32, tag="am")
                nc.vector.tensor_reduce(
                    am[:], x_t[:], axis=AX, op=op.max, apply_absolute_value=True
                )
                ram = smpool.tile([128, 1], F32, tag="ram")
                nc.vector.reciprocal(ram[:], am[:])
                scl = smpool.tile([128, 1], F32, tag="scl")
                nc.vector.tensor_scalar(scl[:], ram[:], QMAX, None, op0=op.mult)
                live[("am", n)] = am
                live[("scl", n)] = scl

            def a_quant(n):
                """single ACT pass: z16 = fp16(x*scl + 1536) - the fp16 cast IS
                the RNE integer rounding."""
                x_t = live.pop(("x", n))
                scl = live.pop(("scl", n))
                xm = xmpool.tile([128, D_IN], FP16, tag="xm")
                nc.scalar.activation(
                    xm[:], x_t[:], Copy, bias=MAGIC16, scale=scl[:]
                )
                live[("q", n)] = xm

            def a_trans(n, drain="split"):
                """PE fp16 transposes + drain (-1536 -> bf16 ints).
                drain: 'split' = [0:512] ACT + [512:1024] DVE (steady),
                'dve' = whole thing on DVE (head, when ACT is slammed)."""
                q = live.pop(("q", n))
                ps_q = psq.tile([128, D_IN], FP16, tag="ps_q")
                for k in range(KT):
                    nc.tensor.transpose(
                        ps_q[:, k * 128:(k + 1) * 128],
                        q[:, k * 128:(k + 1) * 128],
                        ident[:],
                    )
                qT = qtpool.tile([128, D_IN], BF16, tag="qT")
                if drain == "dve":
                    nc.vector.tensor_scalar(
                        qT[:], ps_q[:], -MAGIC16, None, op0=op.add
                    )
                else:
                    nc.scalar.activation(
                        qT[:, 0:512], ps_q[:, 0:512], Copy, bias=-MAGIC16
                    )
                    nc.vector.tensor_scalar(
                        qT[:, 512:1024], ps_q[:, 512:1024], -MAGIC16, None,
                        op0=op.add,
                    )
                live[("qT", n)] = qT

            def b_coef(n):
                """coef = am*s/127 (DVE small)."""
                am = live.pop(("am", n))
                coef = smpool.tile([128, 1], F32, tag="coef")
                nc.vector.tensor_scalar(coef[:], am[:], scoef[:], None, op0=op.mult)
                live[("coef", n)] = coef

            def b_mm_half(n, oh, qT):
                ps = pso.tile([128, 512], F32, tag="ps")
                for k in range(KT):
                    nc.tensor.matmul(
                        ps[:], qT[:, k * 128:(k + 1) * 128],
                        tT[k][:, oh * 512:(oh + 1) * 512],
                        start=(k == 0), stop=(k == KT - 1),
                    )
                live[("ps", n, oh)] = ps

            def b_scale_act(n, out_sb, lo, hi):
                """out = ps*coef on ACT."""
                oh = 0 if lo < 512 else 1
                ps = live[("ps", n, oh)]
                nc.scalar.activation(
                    out_sb[:, lo:hi], ps[:, lo - oh * 512:hi - oh * 512], Copy,
                    scale=live[("coef", n)][:],
                )

            def b_scale_dve(n, out_sb, lo, hi):
                oh = 0 if lo < 512 else 1
                ps = live[("ps", n, oh)]
                nc.vector.tensor_scalar(
                    out_sb[:, lo:hi], ps[:, lo - oh * 512:hi - oh * 512],
                    live[("coef", n)][:], None, op0=op.mult,
                )

            def b_drop(n):
                live.pop(("ps", n, 0))
                live.pop(("ps", n, 1))
                live.pop(("coef", n))

            def b(n, tail=0):
                """full tile: coef/s_adj, both matmul halves, scales, out DMA.
                tail=1: per-half DMAs.  tail=2: final tile - quarter the oh1
                drain across DVE+ACT with separate DMAs for the shortest
                post-matmul chain."""
                b_coef(n)
                qT = live.pop(("qT", n))
                b_mm_half(n, 0, qT)
                out_sb = outpool.tile([128, D_OUT], BF16, tag="osb")
                b_scale_act(n, out_sb, 0, 512)     # runs while oh1 matmuls go
                if tail:
                    nc.sync.dma_start(
                        o_d[n * 128:(n + 1) * 128, 0:512], out_sb[:, 0:512]
                    )
                b_mm_half(n, 1, qT)
                if tail == 2:
                    b_scale_dve(n, out_sb, 512, 768)
                    nc.sync.dma_start(
                        o_d[n * 128:(n + 1) * 128, 512:768], out_sb[:, 512:768]
                    )
                    b_scale_act(n, out_sb, 768, 1024)
                    nc.scalar.dma_start(
                        o_d[n * 128:(n + 1) * 128, 768:1024], out_sb[:, 768:1024]
                    )
                elif tail == 1:
                    b_scale_dve(n, out_sb, 512, 1024)
                    nc.sync.dma_start(
                        o_d[n * 128:(n + 1) * 128, 512:1024], out_sb[:, 512:1024]
                    )
                else:
                    b_scale_dve(n, out_sb, 512, 1024)
                    nc.sync.dma_start(o_d[n * 128:(n + 1) * 128, :], out_sb[:])
                b_drop(n)

            # ---------------- weight phase + x ramp ------------------------
            with (
                tc.tile_pool(name="wpool", bufs=1) as wpool,
                tc.tile_pool(name="wabs", bufs=2) as wabs_pool,
                tc.tile_pool(name="ypool", bufs=4) as ypool,
            ):
                # Rings balanced at 2.5MB each; x0/x1 in the FIRST slots so
                # tiles 0/1 are fully prepped before s lands; w chunks follow
                # so the |W| mean is ready ~1 chunk-time after the last byte.
                w_t = [wpool.tile([128, D_OUT], F32, name=f"w{k}", tag=f"w{k}") for k in range(KT)]
                # ACT table preload first: the load runs while the first DMAs
                # are still in flight (one load covers all afns).
                nc.scalar.activation(preheat[:], ones_col[:], Abs)
                a_dma_issue(0, nc.sync)
                a_dma_issue(1, nc.scalar)
                for k in range(KT):
                    eng = nc.sync if k % 2 == 0 else nc.scalar
                    eng.dma_start(w_t[k][:], w_d[k * 128:(k + 1) * 128, :])
                # x2..x5 queue on the sync ring BEHIND the w chunks: hw FIFO
                # per queue is the only real pacing (scheduler floors do not
                # delay hardware - an independent queue pulls immediately and
                # steals HBM from the weight stream, which slips w7 to ~33us).
                # The sync engine stalls on DMA queue-depth while issuing
                # these, but it has nothing else to do in the head.
                for n5 in (2, 3, 4, 5):
                    a_dma_issue(n5, nc.sync)

                # PE p-state warm-up: per-chunk-gated fp32 transposes keep the
                # PE clocked from the first w byte to the first real matmul
                # (idle PE drops to the 0.65/1.2 GHz p-states).
                for k in range(KT):
                    for j in range(2):
                        ps_wu = psw.tile([128, 128], F32, tag="warm")
                        nc.tensor.transpose(
                            ps_wu[:], w_t[k][:, j * 128:(j + 1) * 128],
                            ident_f32[:],
                        )

                # |wT| chunk sums (ACT even / DVE odd) in arrival order; the
                # last chunk is split ACT/DVE so the mean starts ~0.6us after
                # its last byte.  colsum has 9 slots (k7 uses 7 and 8).
                colsum = wpool.tile([128, KT + 1], F32)

                def cs(k):
                    if k % 2 == 0:
                        wabs = wabs_pool.tile([128, D_OUT], F32, tag="wabs")
                        nc.scalar.activation(
                            wabs[:], w_t[k][:], Abs, accum_out=colsum[:, k:k + 1]
                        )
                    else:
                        nc.vector.tensor_reduce(
                            colsum[:, k:k + 1], w_t[k][:], axis=AX, op=op.add,
                            apply_absolute_value=True,
                        )

                # x0/x1 arrive first: full prep (stats+quant+trans+drain) in
                # the head.  Floors track expected data arrival so the
                # scheduler's static per-engine order matches reality (it
                # otherwise e.g. queues scl0 behind cs1, delaying quant0).
                with tc.tile_wait_until(0.0095):
                    a_stats(0)
                    a_quant(0)
                with tc.tile_wait_until(0.010):
                    a_trans(0, drain="dve")
                with tc.tile_wait_until(0.0105):
                    cs(0)
                with tc.tile_wait_until(0.011):
                    a_stats(1)
                    a_quant(1)
                with tc.tile_wait_until(0.0115):
                    cs(1)
                    a_trans(1, drain="dve")
                for k5 in range(2, KT - 1):
                    with tc.tile_wait_until(0.0105 + 0.00125 * k5):
                        cs(k5)
                with tc.tile_wait_until(0.0105 + 0.00125 * 7):
                    wabs7 = wabs_pool.tile([128, 512], F32, tag="wab7")
                    nc.scalar.activation(
                        wabs7[:], w_t[7][:, 0:512], Abs, accum_out=colsum[:, 7:8]
                    )
                    nc.vector.tensor_reduce(
                        colsum[:, 8:9], w_t[7][:, 512:1024], axis=AX, op=op.add,
                        apply_absolute_value=True,
                    )

                colsum2 = wpool.tile([128, 1], F32)
                nc.vector.tensor_reduce(colsum2[:], colsum[:], axis=AX, op=op.add)
                ps_m1 = psw.tile([1, 2], F32, name="ps_m1", tag="warm")
                nc.tensor.matmul(ps_m1[0:1, 0:1], ones_col[:], colsum2[:])
                pair = wpool.tile([1, 2], F32)
                nc.scalar.activation(pair[:, 0:1], ps_m1[0:1, 0:1], Copy, scale=1.0 / (D_OUT * D_IN))
                nc.vector.reciprocal(pair[:, 1:2], pair[:, 0:1])
                ps_m2 = psw.tile([128, 2], F32, name="ps_m2", tag="warm")
                nc.tensor.matmul(ps_m2[:], ones_row[:], pair[:])
                nc.scalar.copy(bc2[:], ps_m2[:])
                nc.vector.tensor_scalar(scoef[:], bc2[:, 0:1], 1.0 / QMAX, None, op0=op.mult)

                qT0 = live.pop(("qT", 0))
                qT1 = live.pop(("qT", 1))

                # ternary-quantize wT in 512-col halves, oh-major, 2 passes:
                # DVE y = w*(1/s) + MAGIC (single-rounding RNE), then ACT
                # t = Sign(y - MAGIC) in {-1,0,1} (Sign IS the clip since
                # |round(w/s)| <= 2).  k in DVE3OP_KS runs the clip as a
                # 3-op DVE chain instead to balance the engines.
                def wq_half(k, oh):
                    sl = slice(oh * 512, (oh + 1) * 512)
                    y = ypool.tile([128, 512], F32, tag="y")
                    reng = nc.gpsimd if k in GPS_ROUND_KS else nc.vector
                    reng.tensor_scalar(
                        y[:], w_t[k][:, sl], bc2[:, 1:2], MAGIC,
                        op0=op.mult, op1=op.add,
                    )
                    if k in DVE3OP_KS or k in GPS3OP_KS:
                        ceng = nc.gpsimd if k in GPS3OP_KS else nc.vector
                        y2 = ypool.tile([128, 512], F32, tag="y2")
                        ceng.tensor_scalar(
                            y2[:], y[:], MAGIC, 1.0, op0=op.subtract, op1=op.min
                        )
                        ceng.tensor_scalar(
                            tT[k][:, sl], y2[:], -1.0, None, op0=op.max
                        )
                    else:
                        nc.scalar.activation(tT[k][:, sl], y[:], Sign, bias=negm[:])

                for wu in range(6):
                    ps_wu = psw.tile([128, 128], BF16, name=f"wu2_{wu}", tag="warm")
                    nc.tensor.transpose(
                        ps_wu[:], qT0[:, wu * 128:(wu + 1) * 128], ident_bf[:]
                    )
                for k in range(KT):
                    wq_half(k, 0)
                with tc.tile_wait_until(0.021):
                    a_stats(2)
                b_coef(0)
                b_coef(1)
                b_mm_half(0, 0, qT0)
                b_mm_half(1, 0, qT1)
                for k in range(KT):
                    wq_half(k, 1)
                b_mm_half(0, 1, qT0)
                with tc.tile_wait_until(0.0215):
                    a_quant(2)        # ACT, after the oh1 rounds (scl2 ready)
                a_trans(2, drain="dve")   # PE slot between the oh1 halves
                b_mm_half(1, 1, qT1)

                # tiles 0/1 output scales (free the PSUM banks well before
                # tile 2's second matmul half wants them).
                osb0 = outpool.tile([128, D_OUT], BF16, tag="osb")
                b_scale_act(0, osb0, 0, 512)
                osb1 = outpool.tile([128, D_OUT], BF16, tag="osb")
                b_scale_act(1, osb1, 0, 512)
                b_scale_dve(0, osb0, 512, 1024)
                b_drop(0)
                nc.sync.dma_start(o_d[0:128, :], osb0[:])
                b_scale_dve(1, osb1, 512, 1024)
                b_drop(1)
                nc.sync.dma_start(o_d[128:256, :], osb1[:])

                with tc.tile_wait_until(0.0225):
                    a_stats(3)
                    a_quant(3)    # tile 3's transposes happen at loop n=2

            with tc.tile_wait_until(0.026):
                a_stats(4)
                a_quant(4)
                a_stats(5)

            # Per-iteration scheduler floors: the Tile scheduler list-schedules
            # by its own DMA-latency model and otherwise hoists steady-loop
            # work (absmaxes etc.) ahead of the weight mean-chain, stalling s
            # by ~11us.  Floors pin each iteration near its real cadence.
            for n in range(2, TILES):
                with tc.tile_wait_until(0.029 + 0.0039 * (n - 2)):
                    if n + 4 < TILES:
                        a_dma_issue(n + 4)
                    if n + 3 < TILES:
                        a_quant(n + 3)
                    b(n, tail=max(0, n - (TILES - 3)))
                    if n + 4 < TILES:
                        a_stats(n + 4)
                    if n == 2:
                        a_trans(3)
                    if n + 2 < TILES:
                        a_trans(n + 2)

    _split_multiwaits(nc)
    return nc


_NC_CACHE = None


def _get_nc():
    global _NC_CACHE
    if _NC_CACHE is None:
        _NC_CACHE = build_program()
    return _NC_CACHE


def kernel(x: np.ndarray, weight: np.ndarray, trace: bool = False):
    assert x.shape == (B, S, D_IN) and weight.shape == (D_OUT, D_IN)
    nc = _get_nc()
    xf = np.ascontiguousarray(x.reshape(TOKENS, D_IN), dtype=np.float32)
    wT = np.ascontiguousarray(weight.astype(np.float32, copy=False).T)
    in_maps = [
        {
            "x": xf[c * TOK_PER_CORE:(c + 1) * TOK_PER_CORE],
            "wT": wT,
        }
        for c in range(N_CORES)
    ]
    res = run_bass_kernel_spmd(nc, in_maps, core_ids=list(range(N_CORES)), trace=trace)
    kernel.last_results = res
    out = np.concatenate(
        [np.asarray(res.results[c]["out"]).astype(np.float32) for c in range(N_CORES)],
        axis=0,
    )
    return out.reshape(B, S, D_OUT)


kernel.last_results = None


# revision 16
# speedup vs baseline: 1.1604x; 1.1604x over previous
"""BitLinear (BitNet b1.58) Trainium2 kernel, 8-core data-parallel.

Per core (4096 tokens sharded over batch*seq, weight replicated and fed
host-transposed as wT = W.T, a layout/sharding choice):
    q  = round(x*scale)  integers in [-127,127]   -> exact in fp16/bf16
    t  = clip(round(W/s),-1,1) in {-1,0,1}
    out = (q @ t.T) * (absmax*s/127) per token, stored bf16 (host->fp32).

W-quant is 2 passes: DVE y = w*(1/s) + 12582912 (the fp32 magic add
IS the single-rounding RNE to integer - any smaller bias pre-rounds
the fraction at ulp(bias) and flips boundary weights, each of which
corrupts a whole output column), then ACT t = Sign(y - 12582912):
since |round(w/s)| <= 2, Sign IS the clip to {-1,0,1}.  (A biased
bf16(w/s + 192) variant was tried and rejected: double rounding gave
rel err 2.3e-2.  activation accum_out measures PRE-cast values, so it
cannot supply sum(q) either.)

Engine plan per 128-token tile (steady state, PE-bound ~3.9us):
    SP   : x DMA in, out DMA (both on the sync ring - keeps ACT clean)
    DVE  : absmax reduce, 1/absmax, scl, coef, oh1 output scale,
           [512:1024] drain half
    ACT  : z16 = fp16(x*scl + 1536), [0:512] drain half, oh0 out scale
    PE   : 8 fp16 transposes + 16 bf16 matmuls (fp32 PSUM exact)

Weight phase: rings balanced at 2.5MB each (sync: x0,w0,w2,w4,w6,x2,x4;
scalar: x1,w1,w3,w5,w7,x3,x5) so x0/x1 land first and tiles 0/1 are
fully quantized+transposed BEFORE s is ready; first matmul is then
bound only by the first ternary chunk.  Per-chunk-gated PE warmup
transposes keep the clock at max through the whole stream (PE drops to
0.65/1.2 GHz p-states when idle).  ACT table preloaded with a dummy op
(one load covers all afns).  Last |w| colsum split ACT/DVE.  GPSIMD is
never used for bulk elementwise work (measured 14.7us per 1024-elem
op).  DMA-XBAR transpose rejected: ~1200-descriptor storm per tile.
"""

import numpy as np

import concourse.bass as bass
import concourse.mybir as mybir
from concourse import tile, masks
from concourse.bass_utils import run_bass_kernel_spmd

F32 = mybir.dt.float32
BF16 = mybir.dt.bfloat16
FP16 = mybir.dt.float16

N_CORES = 8
B, S, D_IN, D_OUT = 4, 8192, 1024, 1024
TOKENS = B * S                     # 32768
TOK_PER_CORE = TOKENS // N_CORES   # 4096
TILES = TOK_PER_CORE // 128        # 32
KT = D_IN // 128                   # 8 contraction k-chunks

QMAX = 127.0
MAGIC16 = 1536.0     # z16 = fp16(x*scl + 1536): fp16 ulp=1 on [1024,2048)
MAGIC = 12582912.0   # 1.5 * 2**23: fp32 ulp=1 -> the add rounds RNE
# engine balance for W-quant: chunks whose clip runs as a 3-op DVE
# tensor_scalar chain instead of ACT Sign.  (GpSimd was measured at 7.5us
# per 512-col tensor_scalar with min/max - never offload wq there.)
GPS_ROUND_KS = ()
GPS3OP_KS = ()
DVE3OP_KS = (3,)


def _split_multiwaits(nc):
    """walrus here encodes at most ONE sem wait per instruction; Tile's tail
    drain (and occasionally other insts) carry several.  Split extras into
    single-wait NOPs on the same engine, preserving order."""
    for f in nc.m.functions:
        for bb in f.blocks:
            insts = list(bb.instructions)
            if not any(
                i.sync_info and len(i.sync_info.on_wait) > 1 for i in insts
            ):
                continue
            new = []
            for ins in insts:
                si = ins.sync_info
                if si and len(si.on_wait) > 1:
                    waits = list(si.on_wait)
                    for j, w in enumerate(waits[:-1]):
                        nop = mybir.InstNoOp(
                            name=f"{ins.name}_wsp{j}", ins=[], outs=[]
                        )
                        nop.engine = ins.engine
                        nop.sync_info = mybir.SyncInfo(on_wait=[w], on_update=[])
                        new.append(nop)
                    ins.sync_info = mybir.SyncInfo(
                        on_wait=[waits[-1]], on_update=list(si.on_update)
                    )
                new.append(ins)
            bb.instructions = new


def build_program():
    nc = bass.Bass(trn_type="TRN2")
    x_d = nc.dram_tensor("x", [TOK_PER_CORE, D_IN], F32, kind="ExternalInput")
    w_d = nc.dram_tensor("wT", [D_IN, D_OUT], F32, kind="ExternalInput")
    o_d = nc.dram_tensor("out", [TOK_PER_CORE, D_OUT], BF16, kind="ExternalOutput")

    Copy = mybir.ActivationFunctionType.Copy
    Sign = mybir.ActivationFunctionType.Sign
    Abs = mybir.ActivationFunctionType.Abs
    AX = mybir.AxisListType.X
    op = mybir.AluOpType

    with tile.TileContext(nc) as tc:
        from contextlib import ExitStack

        with ExitStack() as ctx:
            singles = ctx.enter_context(tc.tile_pool(name="singles", bufs=1))

            ident = singles.tile([128, 128], FP16)
            masks.make_identity(nc, ident[:])
            ident_f32 = singles.tile([128, 128], F32)
            masks.make_identity(nc, ident_f32[:])
            ident_bf = singles.tile([128, 128], BF16)
            masks.make_identity(nc, ident_bf[:])
            ones_col = singles.tile([128, 1], F32)
            nc.vector.memset(ones_col[:], 1.0)
            ones_row = singles.tile([1, 128], F32)
            nc.vector.memset(ones_row[:], 1.0)
            bc2 = singles.tile([128, 2], F32)    # [s, 1/s] broadcast to 128 parts
            scoef = singles.tile([128, 1], F32)  # s/127 broadcast
            negm = singles.tile([128, 1], F32)   # -MAGIC bias for the Sign pass
            nc.vector.memset(negm[:], -MAGIC)
            preheat = singles.tile([128, 1], F32)

            tT = [singles.tile([128, D_OUT], BF16, name=f"tT{k}", tag=f"tT{k}") for k in range(KT)]

            xpool = ctx.enter_context(tc.tile_pool(name="xpool", bufs=8))
            xmpool = ctx.enter_context(tc.tile_pool(name="xmpool", bufs=3))
            qtpool = ctx.enter_context(tc.tile_pool(name="qtpool", bufs=6))
            outpool = ctx.enter_context(tc.tile_pool(name="outpool", bufs=3))
            smpool = ctx.enter_context(tc.tile_pool(name="smpool", bufs=18))
            psq = ctx.enter_context(tc.tile_pool(name="psq", bufs=2, space="PSUM"))
            pso = ctx.enter_context(tc.tile_pool(name="pso", bufs=5, space="PSUM"))
            psw = ctx.enter_context(tc.tile_pool(name="psw", bufs=1, space="PSUM"))

            live = {}

            def a_dma_issue(n, eng=None):
                """x tile DMA issue only."""
                x_t = xpool.tile([128, D_IN], F32, tag="x")
                (eng or nc.sync).dma_start(x_t[:], x_d[n * 128:(n + 1) * 128, :])
                live[("x", n)] = x_t

            def a_stats(n):
                """per-token absmax/scale smalls (DVE)."""
                x_t = live[("x", n)]
                am = smpool.tile([128, 1], F32, tag="am")
                nc.vector.tensor_reduce(
                    am[:], x_t[:], axis=AX, op=op.max, apply_absolute_value=True
                )
                ram = smpool.tile([128, 1], F32, tag="ram")
                nc.vector.reciprocal(ram[:], am[:])
                scl = smpool.tile([128, 1], F32, tag="scl")
                nc.vector.tensor_scalar(scl[:], ram[:], QMAX, None, op0=op.mult)
                live[("am", n)] = am
                live[("scl", n)] = scl

            def a_quant(n):
                """single ACT pass: z16 = fp16(x*scl + 1536) - the fp16 cast IS
                the RNE integer rounding."""
                x_t = live.pop(("x", n))
                scl = live.pop(("scl", n))
                xm = xmpool.tile([128, D_IN], FP16, tag="xm")
                nc.scalar.activation(
                    xm[:], x_t[:], Copy, bias=MAGIC16, scale=scl[:]
                )
                live[("q", n)] = xm

            def a_trans(n, drain="split"):
                """PE fp16 transposes + drain (-1536 -> bf16 ints).
                drain: 'split' = [0:512] ACT + [512:1024] DVE (steady),
                'dve' = whole thing on DVE (head, when ACT is slammed)."""
                q = live.pop(("q", n))
                ps_q = psq.tile([128, D_IN], FP16, tag="ps_q")
                for k in range(KT):
                    nc.tensor.transpose(
                        ps_q[:, k * 128:(k + 1) * 128],
                        q[:, k * 128:(k + 1) * 128],
                        ident[:],
                    )
                qT = qtpool.tile([128, D_IN], BF16, tag="qT")
                if drain == "dve":
                    nc.vector.tensor_scalar(
                        qT[:], ps_q[:], -MAGIC16, None, op0=op.add
                    )
                else:
                    nc.scalar.activation(
                        qT[:, 0:512], ps_q[:, 0:512], Copy, bias=-MAGIC16
                    )
                    nc.vector.tensor_scalar(
                        qT[:, 512:1024], ps_q[:, 512:1024], -MAGIC16, None,
                        op0=op.add,
                    )
                live[("qT", n)] = qT

            def b_coef(n):
                """coef = am*s/127 (DVE small)."""
                am = live.pop(("am", n))
                coef = smpool.tile([128, 1], F32, tag="coef")
                nc.vector.tensor_scalar(coef[:], am[:], scoef[:], None, op0=op.mult)
                live[("coef", n)] = coef

            def b_mm_half(n, oh, qT):
                ps = pso.tile([128, 512], F32, tag="ps")
                for k in range(KT):
                    nc.tensor.matmul(
                        ps[:], qT[:, k * 128:(k + 1) * 128],
                        tT[k][:, oh * 512:(oh + 1) * 512],
                        start=(k == 0), stop=(k == KT - 1),
                    )
                live[("ps", n, oh)] = ps

            def b_scale_act(n, out_sb, lo, hi):
                """out = ps*coef on ACT."""
                oh = 0 if lo < 512 else 1
                ps = live[("ps", n, oh)]
                nc.scalar.activation(
                    out_sb[:, lo:hi], ps[:, lo - oh * 512:hi - oh * 512], Copy,
                    scale=live[("coef", n)][:],
                )

            def b_scale_dve(n, out_sb, lo, hi):
                oh = 0 if lo < 512 else 1
                ps = live[("ps", n, oh)]
                nc.vector.tensor_scalar(
                    out_sb[:, lo:hi], ps[:, lo - oh * 512:hi - oh * 512],
                    live[("coef", n)][:], None, op0=op.mult,
                )

            def b_drop(n):
                live.pop(("ps", n, 0))
                live.pop(("ps", n, 1))
                live.pop(("coef", n))

            def b(n, tail=0):
                """full tile: coef/s_adj, both matmul halves, scales, out DMA.
                tail=1: per-half DMAs.  tail=2: final tile - quarter the oh1
                drain across DVE+ACT with separate DMAs for the shortest
                post-matmul chain."""
                b_coef(n)
                qT = live.pop(("qT", n))
                b_mm_half(n, 0, qT)
                out_sb = outpool.tile([128, D_OUT], BF16, tag="osb")
                b_scale_act(n, out_sb, 0, 512)     # runs while oh1 matmuls go
                if tail:
                    nc.sync.dma_start(
                        o_d[n * 128:(n + 1) * 128, 0:512], out_sb[:, 0:512]
                    )
                b_mm_half(n, 1, qT)
                if tail == 2:
                    b_scale_dve(n, out_sb, 512, 768)
                    nc.sync.dma_start(
                        o_d[n * 128:(n + 1) * 128, 512:768], out_sb[:, 512:768]
                    )
                    b_scale_act(n, out_sb, 768, 1024)
                    nc.scalar.dma_start(
                        o_d[n * 128:(n + 1) * 128, 768:1024], out_sb[:, 768:1024]
                    )
                elif tail == 1:
                    b_scale_dve(n, out_sb, 512, 1024)
                    nc.sync.dma_start(
                        o_d[n * 128:(n + 1) * 128, 512:1024], out_sb[:, 512:1024]
                    )
                else:
                    b_scale_dve(n, out_sb, 512, 1024)
                    nc.sync.dma_start(o_d[n * 128:(n + 1) * 128, :], out_sb[:])
                b_drop(n)

            # ---------------- weight phase + x ramp ------------------------
            with (
                tc.tile_pool(name="wpool", bufs=1) as wpool,
                tc.tile_pool(name="wabs", bufs=2) as wabs_pool,
                tc.tile_pool(name="ypool", bufs=4) as ypool,
            ):
                # Rings balanced at 2.5MB each; x0/x1 in the FIRST slots so
                # tiles 0/1 are fully prepped before s lands; w chunks follow
                # so the |W| mean is ready ~1 chunk-time after the last byte.
                w_t = [wpool.tile([128, D_OUT], F32, name=f"w{k}", tag=f"w{k}") for k in range(KT)]
                # ACT table preload first: the load runs while the first DMAs
                # are still in flight (one load covers all afns).
                nc.scalar.activation(preheat[:], ones_col[:], Abs)
                a_dma_issue(0, nc.sync)
                a_dma_issue(1, nc.scalar)
                for k in range(KT):
                    eng = nc.sync if k % 2 == 0 else nc.scalar
                    eng.dma_start(w_t[k][:], w_d[k * 128:(k + 1) * 128, :])
                # x2..x5 queue on the sync ring BEHIND the w chunks: hw FIFO
                # per queue is the only real pacing (scheduler floors do not
                # delay hardware - an independent queue pulls immediately and
                # steals HBM from the weight stream, which slips w7 to ~33us).
                # The sync engine stalls on DMA queue-depth while issuing
                # these, but it has nothing else to do in the head.
                for n5 in (2, 3, 4, 5):
                    a_dma_issue(n5, nc.sync)

                # PE p-state warm-up: per-chunk-gated fp32 transposes keep the
                # PE clocked from the first w byte to the first real matmul
                # (idle PE drops to the 0.65/1.2 GHz p-states).
                for k in range(KT):
                    for j in range(2):
                        ps_wu = psw.tile([128, 128], F32, tag="warm")
                        nc.tensor.transpose(
                            ps_wu[:], w_t[k][:, j * 128:(j + 1) * 128],
                            ident_f32[:],
                        )

                # |wT| chunk sums (ACT even / DVE odd) in arrival order; the
                # last chunk is split ACT/DVE so the mean starts ~0.6us after
                # its last byte.  colsum has 9 slots (k7 uses 7 and 8).
                colsum = wpool.tile([128, KT + 1], F32)

                def cs(k):
                    if k % 2 == 0:
                        wabs = wabs_pool.tile([128, D_OUT], F32, tag="wabs")
                        nc.scalar.activation(
                            wabs[:], w_t[k][:], Abs, accum_out=colsum[:, k:k + 1]
                        )
                    else:
                        nc.vector.tensor_reduce(
                            colsum[:, k:k + 1], w_t[k][:], axis=AX, op=op.add,
                            apply_absolute_value=True,
                        )

                # x0/x1 arrive first: full prep (stats+quant+trans+drain) in
                # the head.  Floors track expected data arrival so the
                # scheduler's static per-engine order matches reality (it
                # otherwise e.g. queues scl0 behind cs1, delaying quant0).
                with tc.tile_wait_until(0.0095):
                    a_stats(0)
                    a_quant(0)
                with tc.tile_wait_until(0.010):
                    a_trans(0, drain="dve")
                with tc.tile_wait_until(0.0105):
                    cs(0)
                with tc.tile_wait_until(0.011):
                    a_stats(1)
                    a_quant(1)
                with tc.tile_wait_until(0.0115):
                    cs(1)
                    a_trans(1, drain="dve")
                for k5 in range(2, KT - 1):
                    with tc.tile_wait_until(0.0105 + 0.00125 * k5):
                        cs(k5)
                with tc.tile_wait_until(0.0105 + 0.00125 * 7):
                    wabs7 = wabs_pool.tile([128, 512], F32, tag="wab7")
                    nc.scalar.activation(
                        wabs7[:], w_t[7][:, 0:512], Abs, accum_out=colsum[:, 7:8]
                    )
                    nc.vector.tensor_reduce(
                        colsum[:, 8:9], w_t[7][:, 512:1024], axis=AX, op=op.add,
                        apply_absolute_value=True,
                    )

                colsum2 = wpool.tile([128, 1], F32)
                nc.vector.tensor_reduce(colsum2[:], colsum[:], axis=AX, op=op.add)
                ps_m1 = psw.tile([1, 2], F32, name="ps_m1", tag="warm")
                nc.tensor.matmul(ps_m1[0:1, 0:1], ones_col[:], colsum2[:])
                pair = wpool.tile([1, 2], F32)
                nc.scalar.activation(pair[:, 0:1], ps_m1[0:1, 0:1], Copy, scale=1.0 / (D_OUT * D_IN))
                nc.vector.reciprocal(pair[:, 1:2], pair[:, 0:1])
                ps_m2 = psw.tile([128, 2], F32, name="ps_m2", tag="warm")
                nc.tensor.matmul(ps_m2[:], ones_row[:], pair[:])
                nc.scalar.copy(bc2[:], ps_m2[:])
                nc.vector.tensor_scalar(scoef[:], bc2[:, 0:1], 1.0 / QMAX, None, op0=op.mult)

                qT0 = live.pop(("qT", 0))
                qT1 = live.pop(("qT", 1))

                # ternary-quantize wT in 512-col halves, oh-major, 2 passes:
                # DVE y = w*(1/s) + MAGIC (single-rounding RNE), then ACT
                # t = Sign(y - MAGIC) in {-1,0,1} (Sign IS the clip since
                # |round(w/s)| <= 2).  k in DVE3OP_KS runs the clip as a
                # 3-op DVE chain instead to balance the engines.
                def wq_half(k, oh):
                    sl = slice(oh * 512, (oh + 1) * 512)
                    y = ypool.tile([128, 512], F32, tag="y")
                    reng = nc.gpsimd if k in GPS_ROUND_KS else nc.vector
                    reng.tensor_scalar(
                        y[:], w_t[k][:, sl], bc2[:, 1:2], MAGIC,
                        op0=op.mult, op1=op.add,
                    )
                    if k in DVE3OP_KS or k in GPS3OP_KS:
                        ceng = nc.gpsimd if k in GPS3OP_KS else nc.vector
                        y2 = ypool.tile([128, 512], F32, tag="y2")
                        ceng.tensor_scalar(
                            y2[:], y[:], MAGIC, 1.0, op0=op.subtract, op1=op.min
                        )
                        ceng.tensor_scalar(
                            tT[k][:, sl], y2[:], -1.0, None, op0=op.max
                        )
                    else:
                        nc.scalar.activation(tT[k][:, sl], y[:], Sign, bias=negm[:])

                for wu in range(6):
                    ps_wu = psw.tile([128, 128], BF16, name=f"wu2_{wu}", tag="warm")
                    nc.tensor.transpose(
                        ps_wu[:], qT0[:, wu * 128:(wu + 1) * 128], ident_bf[:]
                    )
                for k in range(KT):
                    wq_half(k, 0)
                with tc.tile_wait_until(0.021):
                    a_stats(2)
                b_coef(0)
                b_coef(1)
                b_mm_half(0, 0, qT0)
                b_mm_half(1, 0, qT1)
                for k in range(KT):
                    wq_half(k, 1)
                b_mm_half(0, 1, qT0)
                with tc.tile_wait_until(0.0215):
                    a_quant(2)        # ACT, after the oh1 rounds (scl2 ready)
                a_trans(2, drain="dve")   # PE slot between the oh1 halves
                b_mm_half(1, 1, qT1)

                # tiles 0/1 output scales (free the PSUM banks well before
                # tile 2's second matmul half wants them).
                osb0 = outpool.tile([128, D_OUT], BF16, tag="osb")
                b_scale_act(0, osb0, 0, 512)
                osb1 = outpool.tile([128, D_OUT], BF16, tag="osb")
                b_scale_act(1, osb1, 0, 512)
                b_scale_dve(0, osb0, 512, 1024)
                b_drop(0)
                nc.sync.dma_start(o_d[0:128, :], osb0[:])
                b_scale_dve(1, osb1, 512, 1024)
                b_drop(1)
                nc.sync.dma_start(o_d[128:256, :], osb1[:])

                with tc.tile_wait_until(0.0225):
                    a_stats(3)
                    a_quant(3)    # tile 3's transposes happen at loop n=2

            with tc.tile_wait_until(0.026):
                a_stats(4)
                a_quant(4)
                a_stats(5)

            # Per-iteration scheduler floors: the Tile scheduler list-schedules
            # by its own DMA-latency model and otherwise hoists steady-loop
            # work (absmaxes etc.) ahead of the weight mean-chain, stalling s
            # by ~11us.  Floors pin each iteration near its real cadence.
            for n in range(2, TILES):
                with tc.tile_wait_until(0.029 + 0.0039 * (n - 2)):
                    if n + 4 < TILES:
                        a_dma_issue(n + 4)
                    if n + 3 < TILES:
                        a_quant(n + 3)
                    b(n, tail=max(0, n - (TILES - 3)))
                    if n + 4 < TILES:
                        a_stats(n + 4)
                    if n == 2:
                        a_trans(3)
                    if n + 2 < TILES:
                        a_trans(n + 2)

    _split_multiwaits(nc)
    return nc


_NC_CACHE = None


def _get_nc():
    global _NC_CACHE
    if _NC_CACHE is None:
        _NC_CACHE = build_program()
    return _NC_CACHE


def kernel(x: np.ndarray, weight: np.ndarray, trace: bool = False):
    assert x.shape == (B, S, D_IN) and weight.shape == (D_OUT, D_IN)
    nc = _get_nc()
    xf = np.ascontiguousarray(x.reshape(TOKENS, D_IN), dtype=np.float32)
    wT = np.ascontiguousarray(weight.astype(np.float32, copy=False).T)
    in_maps = [
        {
            "x": xf[c * TOK_PER_CORE:(c + 1) * TOK_PER_CORE],
            "wT": wT,
        }
        for c in range(N_CORES)
    ]
    res = run_bass_kernel_spmd(nc, in_maps, core_ids=list(range(N_CORES)), trace=trace)
    kernel.last_results = res
    out = np.concatenate(
        [np.asarray(res.results[c]["out"]).astype(np.float32) for c in range(N_CORES)],
        axis=0,
    )
    return out.reshape(B, S, D_OUT)


kernel.last_results = None


# revision 17
# speedup vs baseline: 1.1651x; 1.0041x over previous
"""BitLinear (BitNet b1.58) Trainium2 kernel, 8-core data-parallel.

Per core (4096 tokens sharded over batch*seq, weight replicated and fed
host-transposed as wT = W.T, a layout/sharding choice):
    q  = round(x*scale)  integers in [-127,127]   -> exact in fp16/bf16
    t  = clip(round(W/s),-1,1) in {-1,0,1}
    out = (q @ t.T) * (absmax*s/127) per token, stored bf16 (host->fp32).

W-quant is 2 passes: DVE y = w*(1/s) + 12582912 (the fp32 magic add
IS the single-rounding RNE to integer - any smaller bias pre-rounds
the fraction at ulp(bias) and flips boundary weights, each of which
corrupts a whole output column), then ACT t = Sign(y - 12582912):
since |round(w/s)| <= 2, Sign IS the clip to {-1,0,1}.  (A biased
bf16(w/s + 192) variant was tried and rejected: double rounding gave
rel err 2.3e-2.  activation accum_out measures PRE-cast values, so it
cannot supply sum(q) either.)

Engine plan per 128-token tile (steady state, PE-bound ~3.9us):
    SP   : x DMA in, out DMA (both on the sync ring - keeps ACT clean)
    DVE  : absmax reduce, 1/absmax, scl, coef, oh1 output scale,
           [512:1024] drain half
    ACT  : z16 = fp16(x*scl + 1536), [0:512] drain half, oh0 out scale
    PE   : 8 fp16 transposes + 16 bf16 matmuls (fp32 PSUM exact)

Weight phase: rings balanced at 2.5MB each (sync: x0,w0,w2,w4,w6,x2,x4;
scalar: x1,w1,w3,w5,w7,x3,x5) so x0/x1 land first and tiles 0/1 are
fully quantized+transposed BEFORE s is ready; first matmul is then
bound only by the first ternary chunk.  Per-chunk-gated PE warmup
transposes keep the clock at max through the whole stream (PE drops to
0.65/1.2 GHz p-states when idle).  ACT table preloaded with a dummy op
(one load covers all afns).  Last |w| colsum split ACT/DVE.  GPSIMD is
never used for bulk elementwise work (measured 14.7us per 1024-elem
op).  DMA-XBAR transpose rejected: ~1200-descriptor storm per tile.
"""

import numpy as np

import concourse.bass as bass
import concourse.mybir as mybir
from concourse import tile, masks
from concourse.bass_utils import run_bass_kernel_spmd

F32 = mybir.dt.float32
BF16 = mybir.dt.bfloat16
FP16 = mybir.dt.float16

N_CORES = 8
B, S, D_IN, D_OUT = 4, 8192, 1024, 1024
TOKENS = B * S                     # 32768
TOK_PER_CORE = TOKENS // N_CORES   # 4096
TILES = TOK_PER_CORE // 128        # 32
KT = D_IN // 128                   # 8 contraction k-chunks

QMAX = 127.0
MAGIC16 = 1536.0     # z16 = fp16(x*scl + 1536): fp16 ulp=1 on [1024,2048)
MAGIC = 12582912.0   # 1.5 * 2**23: fp32 ulp=1 -> the add rounds RNE
# engine balance for W-quant: chunks whose clip runs as a 3-op DVE
# tensor_scalar chain instead of ACT Sign.  (GpSimd was measured at 7.5us
# per 512-col tensor_scalar with min/max - never offload wq there.)
GPS_ROUND_KS = ()
GPS3OP_KS = ()
DVE3OP_KS = (3,)


def _split_multiwaits(nc):
    """walrus here encodes at most ONE sem wait per instruction; Tile's tail
    drain (and occasionally other insts) carry several.  Split extras into
    single-wait NOPs on the same engine, preserving order."""
    for f in nc.m.functions:
        for bb in f.blocks:
            insts = list(bb.instructions)
            if not any(
                i.sync_info and len(i.sync_info.on_wait) > 1 for i in insts
            ):
                continue
            new = []
            for ins in insts:
                si = ins.sync_info
                if si and len(si.on_wait) > 1:
                    waits = list(si.on_wait)
                    for j, w in enumerate(waits[:-1]):
                        nop = mybir.InstNoOp(
                            name=f"{ins.name}_wsp{j}", ins=[], outs=[]
                        )
                        nop.engine = ins.engine
                        nop.sync_info = mybir.SyncInfo(on_wait=[w], on_update=[])
                        new.append(nop)
                    ins.sync_info = mybir.SyncInfo(
                        on_wait=[waits[-1]], on_update=list(si.on_update)
                    )
                new.append(ins)
            bb.instructions = new


def build_program():
    nc = bass.Bass(trn_type="TRN2")
    x_d = nc.dram_tensor("x", [TOK_PER_CORE, D_IN], F32, kind="ExternalInput")
    w_d = nc.dram_tensor("wT", [D_IN, D_OUT], F32, kind="ExternalInput")
    o_d = nc.dram_tensor("out", [TOK_PER_CORE, D_OUT], BF16, kind="ExternalOutput")

    Copy = mybir.ActivationFunctionType.Copy
    Sign = mybir.ActivationFunctionType.Sign
    Abs = mybir.ActivationFunctionType.Abs
    AX = mybir.AxisListType.X
    op = mybir.AluOpType

    with tile.TileContext(nc) as tc:
        from contextlib import ExitStack

        with ExitStack() as ctx:
            singles = ctx.enter_context(tc.tile_pool(name="singles", bufs=1))

            ident = singles.tile([128, 128], FP16)
            masks.make_identity(nc, ident[:])
            ident_f32 = singles.tile([128, 128], F32)
            masks.make_identity(nc, ident_f32[:])
            ident_bf = singles.tile([128, 128], BF16)
            masks.make_identity(nc, ident_bf[:])
            ones_col = singles.tile([128, 1], F32)
            nc.vector.memset(ones_col[:], 1.0)
            ones_row = singles.tile([1, 128], F32)
            nc.vector.memset(ones_row[:], 1.0)
            bc2 = singles.tile([128, 2], F32)    # [s, 1/s] broadcast to 128 parts
            scoef = singles.tile([128, 1], F32)  # s/127 broadcast
            negm = singles.tile([128, 1], F32)   # -MAGIC bias for the Sign pass
            nc.vector.memset(negm[:], -MAGIC)
            preheat = singles.tile([128, 1], F32)

            tT = [singles.tile([128, D_OUT], BF16, name=f"tT{k}", tag=f"tT{k}") for k in range(KT)]

            xpool = ctx.enter_context(tc.tile_pool(name="xpool", bufs=8))
            xmpool = ctx.enter_context(tc.tile_pool(name="xmpool", bufs=3))
            qtpool = ctx.enter_context(tc.tile_pool(name="qtpool", bufs=6))
            outpool = ctx.enter_context(tc.tile_pool(name="outpool", bufs=3))
            smpool = ctx.enter_context(tc.tile_pool(name="smpool", bufs=18))
            psq = ctx.enter_context(tc.tile_pool(name="psq", bufs=2, space="PSUM"))
            pso = ctx.enter_context(tc.tile_pool(name="pso", bufs=5, space="PSUM"))
            psw = ctx.enter_context(tc.tile_pool(name="psw", bufs=1, space="PSUM"))

            live = {}

            def a_dma_issue(n, eng=None):
                """x tile DMA issue only."""
                x_t = xpool.tile([128, D_IN], F32, tag="x")
                (eng or nc.sync).dma_start(x_t[:], x_d[n * 128:(n + 1) * 128, :])
                live[("x", n)] = x_t

            def a_stats(n):
                """per-token absmax/scale smalls (DVE)."""
                x_t = live[("x", n)]
                am = smpool.tile([128, 1], F32, tag="am")
                nc.vector.tensor_reduce(
                    am[:], x_t[:], axis=AX, op=op.max, apply_absolute_value=True
                )
                ram = smpool.tile([128, 1], F32, tag="ram")
                nc.vector.reciprocal(ram[:], am[:])
                scl = smpool.tile([128, 1], F32, tag="scl")
                nc.vector.tensor_scalar(scl[:], ram[:], QMAX, None, op0=op.mult)
                live[("am", n)] = am
                live[("scl", n)] = scl

            def a_quant(n):
                """single ACT pass: z16 = fp16(x*scl + 1536) - the fp16 cast IS
                the RNE integer rounding."""
                x_t = live.pop(("x", n))
                scl = live.pop(("scl", n))
                xm = xmpool.tile([128, D_IN], FP16, tag="xm")
                nc.scalar.activation(
                    xm[:], x_t[:], Copy, bias=MAGIC16, scale=scl[:]
                )
                live[("q", n)] = xm

            def a_trans(n, drain="split"):
                """PE fp16 transposes + drain (-1536 -> bf16 ints).
                drain: 'split' = [0:512] ACT + [512:1024] DVE (steady),
                'dve' = whole thing on DVE (head, when ACT is slammed)."""
                q = live.pop(("q", n))
                ps_q = psq.tile([128, D_IN], FP16, tag="ps_q")
                for k in range(KT):
                    nc.tensor.transpose(
                        ps_q[:, k * 128:(k + 1) * 128],
                        q[:, k * 128:(k + 1) * 128],
                        ident[:],
                    )
                qT = qtpool.tile([128, D_IN], BF16, tag="qT")
                if drain == "dve":
                    nc.vector.tensor_scalar(
                        qT[:], ps_q[:], -MAGIC16, None, op0=op.add
                    )
                else:
                    nc.scalar.activation(
                        qT[:, 0:512], ps_q[:, 0:512], Copy, bias=-MAGIC16
                    )
                    nc.vector.tensor_scalar(
                        qT[:, 512:1024], ps_q[:, 512:1024], -MAGIC16, None,
                        op0=op.add,
                    )
                live[("qT", n)] = qT

            def b_coef(n):
                """coef = am*s/127 (DVE small)."""
                am = live.pop(("am", n))
                coef = smpool.tile([128, 1], F32, tag="coef")
                nc.vector.tensor_scalar(coef[:], am[:], scoef[:], None, op0=op.mult)
                live[("coef", n)] = coef

            def b_mm_half(n, oh, qT):
                ps = pso.tile([128, 512], F32, tag="ps")
                for k in range(KT):
                    nc.tensor.matmul(
                        ps[:], qT[:, k * 128:(k + 1) * 128],
                        tT[k][:, oh * 512:(oh + 1) * 512],
                        start=(k == 0), stop=(k == KT - 1),
                    )
                live[("ps", n, oh)] = ps

            def b_scale_act(n, out_sb, lo, hi):
                """out = ps*coef on ACT."""
                oh = 0 if lo < 512 else 1
                ps = live[("ps", n, oh)]
                nc.scalar.activation(
                    out_sb[:, lo:hi], ps[:, lo - oh * 512:hi - oh * 512], Copy,
                    scale=live[("coef", n)][:],
                )

            def b_scale_dve(n, out_sb, lo, hi):
                oh = 0 if lo < 512 else 1
                ps = live[("ps", n, oh)]
                nc.vector.tensor_scalar(
                    out_sb[:, lo:hi], ps[:, lo - oh * 512:hi - oh * 512],
                    live[("coef", n)][:], None, op0=op.mult,
                )

            def b_drop(n):
                live.pop(("ps", n, 0))
                live.pop(("ps", n, 1))
                live.pop(("coef", n))

            def b(n, tail=0):
                """full tile: coef/s_adj, both matmul halves, scales, out DMA.
                tail=1: per-half DMAs.  tail=2: final tile - quarter the oh1
                drain across DVE+ACT with separate DMAs for the shortest
                post-matmul chain."""
                b_coef(n)
                qT = live.pop(("qT", n))
                b_mm_half(n, 0, qT)
                out_sb = outpool.tile([128, D_OUT], BF16, tag="osb")
                b_scale_act(n, out_sb, 0, 512)     # runs while oh1 matmuls go
                if tail:
                    nc.sync.dma_start(
                        o_d[n * 128:(n + 1) * 128, 0:512], out_sb[:, 0:512]
                    )
                b_mm_half(n, 1, qT)
                if tail == 2:
                    b_scale_dve(n, out_sb, 512, 768)
                    nc.sync.dma_start(
                        o_d[n * 128:(n + 1) * 128, 512:768], out_sb[:, 512:768]
                    )
                    b_scale_act(n, out_sb, 768, 1024)
                    nc.scalar.dma_start(
                        o_d[n * 128:(n + 1) * 128, 768:1024], out_sb[:, 768:1024]
                    )
                elif tail == 1:
                    b_scale_dve(n, out_sb, 512, 1024)
                    nc.sync.dma_start(
                        o_d[n * 128:(n + 1) * 128, 512:1024], out_sb[:, 512:1024]
                    )
                else:
                    b_scale_dve(n, out_sb, 512, 1024)
                    nc.sync.dma_start(o_d[n * 128:(n + 1) * 128, :], out_sb[:])
                b_drop(n)

            # ---------------- weight phase + x ramp ------------------------
            with (
                tc.tile_pool(name="wpool", bufs=1) as wpool,
                tc.tile_pool(name="wabs", bufs=2) as wabs_pool,
                tc.tile_pool(name="ypool", bufs=4) as ypool,
            ):
                # Rings balanced at 2.5MB each; x0/x1 in the FIRST slots so
                # tiles 0/1 are fully prepped before s lands; w chunks follow
                # so the |W| mean is ready ~1 chunk-time after the last byte.
                w_t = [wpool.tile([128, D_OUT], F32, name=f"w{k}", tag=f"w{k}") for k in range(KT)]
                a_dma_issue(0, nc.sync)
                a_dma_issue(1, nc.scalar)
                for k in range(2):
                    eng = nc.sync if k % 2 == 0 else nc.scalar
                    eng.dma_start(w_t[k][:], w_d[k * 128:(k + 1) * 128, :])
                # ACT table preload (one load covers all afns): after w1's
                # issue so it delays no transfer (the queue is busy with
                # x1+w1 anyway), but early enough to be done before cs(0).
                nc.scalar.activation(preheat[:], ones_col[:], Abs)
                for k in range(2, KT):
                    eng = nc.sync if k % 2 == 0 else nc.scalar
                    eng.dma_start(w_t[k][:], w_d[k * 128:(k + 1) * 128, :])
                # x2..x5 queue on the sync ring BEHIND the w chunks: hw FIFO
                # per queue is the only real pacing (scheduler floors do not
                # delay hardware - an independent queue pulls immediately and
                # steals HBM from the weight stream, which slips w7 to ~33us).
                # The sync engine stalls on DMA queue-depth while issuing
                # these, but it has nothing else to do in the head.
                for n5 in (2, 3, 4, 5):
                    a_dma_issue(n5, nc.sync)

                # PE p-state warm-up: per-chunk-gated fp32 transposes keep the
                # PE clocked from the first w byte to the first real matmul
                # (idle PE drops to the 0.65/1.2 GHz p-states).
                for k in range(KT):
                    for j in range(2):
                        ps_wu = psw.tile([128, 128], F32, tag="warm")
                        nc.tensor.transpose(
                            ps_wu[:], w_t[k][:, j * 128:(j + 1) * 128],
                            ident_f32[:],
                        )

                # |wT| chunk sums (ACT even / DVE odd) in arrival order; the
                # last chunk is split ACT/DVE so the mean starts ~0.6us after
                # its last byte.  colsum has 9 slots (k7 uses 7 and 8).
                colsum = wpool.tile([128, KT + 1], F32)

                def cs(k):
                    if k % 2 == 0:
                        wabs = wabs_pool.tile([128, D_OUT], F32, tag="wabs")
                        nc.scalar.activation(
                            wabs[:], w_t[k][:], Abs, accum_out=colsum[:, k:k + 1]
                        )
                    else:
                        nc.vector.tensor_reduce(
                            colsum[:, k:k + 1], w_t[k][:], axis=AX, op=op.add,
                            apply_absolute_value=True,
                        )

                # x0/x1 arrive first: full prep (stats+quant+trans+drain) in
                # the head.  Floors track expected data arrival so the
                # scheduler's static per-engine order matches reality (it
                # otherwise e.g. queues scl0 behind cs1, delaying quant0).
                with tc.tile_wait_until(0.0095):
                    a_stats(0)
                    a_quant(0)
                with tc.tile_wait_until(0.010):
                    a_trans(0, drain="dve")
                with tc.tile_wait_until(0.0105):
                    cs(0)
                with tc.tile_wait_until(0.011):
                    a_stats(1)
                    a_quant(1)
                with tc.tile_wait_until(0.0115):
                    cs(1)
                    a_trans(1, drain="dve")
                for k5 in range(2, KT - 1):
                    with tc.tile_wait_until(0.0105 + 0.00125 * k5):
                        cs(k5)
                with tc.tile_wait_until(0.0105 + 0.00125 * 7):
                    wabs7 = wabs_pool.tile([128, 512], F32, tag="wab7")
                    nc.scalar.activation(
                        wabs7[:], w_t[7][:, 0:512], Abs, accum_out=colsum[:, 7:8]
                    )
                    nc.vector.tensor_reduce(
                        colsum[:, 8:9], w_t[7][:, 512:1024], axis=AX, op=op.add,
                        apply_absolute_value=True,
                    )

                colsum2 = wpool.tile([128, 1], F32)
                nc.vector.tensor_reduce(colsum2[:], colsum[:], axis=AX, op=op.add)
                ps_m1 = psw.tile([1, 2], F32, name="ps_m1", tag="warm")
                nc.tensor.matmul(ps_m1[0:1, 0:1], ones_col[:], colsum2[:])
                pair = wpool.tile([1, 2], F32)
                nc.scalar.activation(pair[:, 0:1], ps_m1[0:1, 0:1], Copy, scale=1.0 / (D_OUT * D_IN))
                nc.vector.reciprocal(pair[:, 1:2], pair[:, 0:1])
                ps_m2 = psw.tile([128, 2], F32, name="ps_m2", tag="warm")
                nc.tensor.matmul(ps_m2[:], ones_row[:], pair[:])
                nc.scalar.copy(bc2[:], ps_m2[:])
                nc.vector.tensor_scalar(scoef[:], bc2[:, 0:1], 1.0 / QMAX, None, op0=op.mult)

                qT0 = live.pop(("qT", 0))
                qT1 = live.pop(("qT", 1))

                # ternary-quantize wT in 512-col halves, oh-major, 2 passes:
                # DVE y = w*(1/s) + MAGIC (single-rounding RNE), then ACT
                # t = Sign(y - MAGIC) in {-1,0,1} (Sign IS the clip since
                # |round(w/s)| <= 2).  k in DVE3OP_KS runs the clip as a
                # 3-op DVE chain instead to balance the engines.
                def wq_half(k, oh):
                    sl = slice(oh * 512, (oh + 1) * 512)
                    y = ypool.tile([128, 512], F32, tag="y")
                    reng = nc.gpsimd if k in GPS_ROUND_KS else nc.vector
                    reng.tensor_scalar(
                        y[:], w_t[k][:, sl], bc2[:, 1:2], MAGIC,
                        op0=op.mult, op1=op.add,
                    )
                    if k in DVE3OP_KS or k in GPS3OP_KS:
                        ceng = nc.gpsimd if k in GPS3OP_KS else nc.vector
                        y2 = ypool.tile([128, 512], F32, tag="y2")
                        ceng.tensor_scalar(
                            y2[:], y[:], MAGIC, 1.0, op0=op.subtract, op1=op.min
                        )
                        ceng.tensor_scalar(
                            tT[k][:, sl], y2[:], -1.0, None, op0=op.max
                        )
                    else:
                        nc.scalar.activation(tT[k][:, sl], y[:], Sign, bias=negm[:])

                for wu in range(6):
                    ps_wu = psw.tile([128, 128], BF16, name=f"wu2_{wu}", tag="warm")
                    nc.tensor.transpose(
                        ps_wu[:], qT0[:, wu * 128:(wu + 1) * 128], ident_bf[:]
                    )
                for k in range(KT):
                    wq_half(k, 0)
                with tc.tile_wait_until(0.021):
                    a_stats(2)
                b_coef(0)
                b_coef(1)
                b_mm_half(0, 0, qT0)
                b_mm_half(1, 0, qT1)
                # dep-free fillers: hold the PE p-state through the wait for
                # the oh1 ternary halves (any >0.3us idle drops the clock and
                # the next ~3us of matmuls run at the 1.2GHz p-state).
                for wu in range(20):
                    ps_wu = psw.tile([128, 128], BF16, name=f"wu3_{wu}", tag="warm")
                    nc.tensor.transpose(ps_wu[:], ident_bf[:], ident_bf[:])
                for k in range(KT):
                    wq_half(k, 1)
                b_mm_half(0, 1, qT0)
                with tc.tile_wait_until(0.0215):
                    a_quant(2)        # ACT, after the oh1 rounds (scl2 ready)
                a_trans(2, drain="dve")   # PE slot between the oh1 halves
                b_mm_half(1, 1, qT1)

                with tc.tile_wait_until(0.0225):
                    a_stats(3)
                    a_quant(3)    # tile 3's transposes happen at loop n=2

            with tc.tile_wait_until(0.024):
                a_stats(4)
                a_quant(4)
                a_stats(5)

            # Per-iteration scheduler floors: the Tile scheduler list-schedules
            # by its own DMA-latency model and otherwise hoists steady-loop
            # work (absmaxes etc.) ahead of the weight mean-chain, stalling s
            # by ~11us.  Floors pin each iteration near its real cadence.
            for n in range(2, TILES):
                with tc.tile_wait_until(0.025 + 0.0039 * (n - 2)):
                    if n in (2, 3):
                        # tiles 0/1 output scales ride here: their PSUM banks
                        # are only needed by tile n's second matmul half.
                        n01 = n - 2
                        osb01 = outpool.tile([128, D_OUT], BF16, tag="osb")
                        b_scale_act(n01, osb01, 0, 512)
                        b_scale_dve(n01, osb01, 512, 1024)
                        b_drop(n01)
                        nc.sync.dma_start(
                            o_d[n01 * 128:(n01 + 1) * 128, :], osb01[:]
                        )
                    if n + 4 < TILES:
                        a_dma_issue(n + 4)
                    if n + 3 < TILES:
                        a_quant(n + 3)
                    b(n, tail=max(0, n - (TILES - 3)))
                    if n + 4 < TILES:
                        a_stats(n + 4)
                    if n == 2:
                        a_trans(3)
                    if n + 2 < TILES:
                        a_trans(n + 2)

    _split_multiwaits(nc)
    return nc


_NC_CACHE = None


def _get_nc():
    global _NC_CACHE
    if _NC_CACHE is None:
        _NC_CACHE = build_program()
    return _NC_CACHE


def kernel(x: np.ndarray, weight: np.ndarray, trace: bool = False):
    assert x.shape == (B, S, D_IN) and weight.shape == (D_OUT, D_IN)
    nc = _get_nc()
    xf = np.ascontiguousarray(x.reshape(TOKENS, D_IN), dtype=np.float32)
    wT = np.ascontiguousarray(weight.astype(np.float32, copy=False).T)
    in_maps = [
        {
            "x": xf[c * TOK_PER_CORE:(c + 1) * TOK_PER_CORE],
            "wT": wT,
        }
        for c in range(N_CORES)
    ]
    res = run_bass_kernel_spmd(nc, in_maps, core_ids=list(range(N_CORES)), trace=trace)
    kernel.last_results = res
    out = np.concatenate(
        [np.asarray(res.results[c]["out"]).astype(np.float32) for c in range(N_CORES)],
        axis=0,
    )
    return out.reshape(B, S, D_OUT)


kernel.last_results = None


# revision 18
# speedup vs baseline: 1.1694x; 1.0037x over previous
"""BitLinear (BitNet b1.58) Trainium2 kernel, 8-core data-parallel.

Per core (4096 tokens sharded over batch*seq, weight replicated and fed
host-transposed as wT = W.T, a layout/sharding choice):
    q  = round(x*scale)  integers in [-127,127]   -> exact in fp16/bf16
    t  = clip(round(W/s),-1,1) in {-1,0,1}
    out = (q @ t.T) * (absmax*s/127) per token, stored bf16 (host->fp32).

W-quant is 2 passes: DVE y = w*(1/s) + 12582912 (the fp32 magic add
IS the single-rounding RNE to integer - any smaller bias pre-rounds
the fraction at ulp(bias) and flips boundary weights, each of which
corrupts a whole output column), then ACT t = Sign(y - 12582912):
since |round(w/s)| <= 2, Sign IS the clip to {-1,0,1}.  (A biased
bf16(w/s + 192) variant was tried and rejected: double rounding gave
rel err 2.3e-2.  activation accum_out measures PRE-cast values, so it
cannot supply sum(q) either.)

Engine plan per 128-token tile (steady state, PE-bound ~3.9us):
    SP   : x DMA in, out DMA (both on the sync ring - keeps ACT clean)
    DVE  : absmax reduce, 1/absmax, scl, coef, oh1 output scale,
           [512:1024] drain half
    ACT  : z16 = fp16(x*scl + 1536), [0:512] drain half, oh0 out scale
    PE   : 8 fp16 transposes + 16 bf16 matmuls (fp32 PSUM exact)

Weight phase: rings balanced at 2.5MB each (sync: x0,w0,w2,w4,w6,x2,x4;
scalar: x1,w1,w3,w5,w7,x3,x5) so x0/x1 land first and tiles 0/1 are
fully quantized+transposed BEFORE s is ready; first matmul is then
bound only by the first ternary chunk.  Per-chunk-gated PE warmup
transposes keep the clock at max through the whole stream (PE drops to
0.65/1.2 GHz p-states when idle).  ACT table preloaded with a dummy op
(one load covers all afns).  Last |w| colsum split ACT/DVE.  GPSIMD is
never used for bulk elementwise work (measured 14.7us per 1024-elem
op).  DMA-XBAR transpose rejected: ~1200-descriptor storm per tile.
"""

import numpy as np

import concourse.bass as bass
import concourse.mybir as mybir
from concourse import tile, masks
from concourse.bass_utils import run_bass_kernel_spmd

F32 = mybir.dt.float32
BF16 = mybir.dt.bfloat16
FP16 = mybir.dt.float16

N_CORES = 8
B, S, D_IN, D_OUT = 4, 8192, 1024, 1024
TOKENS = B * S                     # 32768
TOK_PER_CORE = TOKENS // N_CORES   # 4096
TILES = TOK_PER_CORE // 128        # 32
KT = D_IN // 128                   # 8 contraction k-chunks

QMAX = 127.0
MAGIC16 = 1536.0     # z16 = fp16(x*scl + 1536): fp16 ulp=1 on [1024,2048)
MAGIC = 12582912.0   # 1.5 * 2**23: fp32 ulp=1 -> the add rounds RNE
# engine balance for W-quant: chunks whose clip runs as a 3-op DVE
# tensor_scalar chain instead of ACT Sign.  (GpSimd was measured at 7.5us
# per 512-col tensor_scalar with min/max - never offload wq there.)
GPS_ROUND_KS = ()
GPS3OP_KS = ()
DVE3OP_KS = (3,)


def _split_multiwaits(nc):
    """walrus here encodes at most ONE sem wait per instruction; Tile's tail
    drain (and occasionally other insts) carry several.  Split extras into
    single-wait NOPs on the same engine, preserving order."""
    for f in nc.m.functions:
        for bb in f.blocks:
            insts = list(bb.instructions)
            if not any(
                i.sync_info and len(i.sync_info.on_wait) > 1 for i in insts
            ):
                continue
            new = []
            for ins in insts:
                si = ins.sync_info
                if si and len(si.on_wait) > 1:
                    waits = list(si.on_wait)
                    for j, w in enumerate(waits[:-1]):
                        nop = mybir.InstNoOp(
                            name=f"{ins.name}_wsp{j}", ins=[], outs=[]
                        )
                        nop.engine = ins.engine
                        nop.sync_info = mybir.SyncInfo(on_wait=[w], on_update=[])
                        new.append(nop)
                    ins.sync_info = mybir.SyncInfo(
                        on_wait=[waits[-1]], on_update=list(si.on_update)
                    )
                new.append(ins)
            bb.instructions = new


def build_program():
    nc = bass.Bass(trn_type="TRN2")
    x_d = nc.dram_tensor("x", [TOK_PER_CORE, D_IN], F32, kind="ExternalInput")
    w_d = nc.dram_tensor("wT", [D_IN, D_OUT], F32, kind="ExternalInput")
    o_d = nc.dram_tensor("out", [TOK_PER_CORE, D_OUT], BF16, kind="ExternalOutput")

    Copy = mybir.ActivationFunctionType.Copy
    Sign = mybir.ActivationFunctionType.Sign
    Abs = mybir.ActivationFunctionType.Abs
    AX = mybir.AxisListType.X
    op = mybir.AluOpType

    with tile.TileContext(nc) as tc:
        from contextlib import ExitStack

        with ExitStack() as ctx:
            singles = ctx.enter_context(tc.tile_pool(name="singles", bufs=1))

            ident = singles.tile([128, 128], FP16)
            masks.make_identity(nc, ident[:])
            ident_f32 = singles.tile([128, 128], F32)
            masks.make_identity(nc, ident_f32[:])
            ident_bf = singles.tile([128, 128], BF16)
            masks.make_identity(nc, ident_bf[:])
            ones_col = singles.tile([128, 1], F32)
            nc.vector.memset(ones_col[:], 1.0)
            ones_row = singles.tile([1, 128], F32)
            nc.vector.memset(ones_row[:], 1.0)
            bc2 = singles.tile([128, 2], F32)    # [s, 1/s] broadcast to 128 parts
            scoef = singles.tile([128, 1], F32)  # s/127 broadcast
            negm = singles.tile([128, 1], F32)   # -MAGIC bias for the Sign pass
            nc.vector.memset(negm[:], -MAGIC)
            preheat = singles.tile([128, 1], F32)

            tT = [singles.tile([128, D_OUT], BF16, name=f"tT{k}", tag=f"tT{k}") for k in range(KT)]

            xpool = ctx.enter_context(tc.tile_pool(name="xpool", bufs=8))
            xmpool = ctx.enter_context(tc.tile_pool(name="xmpool", bufs=3))
            qtpool = ctx.enter_context(tc.tile_pool(name="qtpool", bufs=6))
            outpool = ctx.enter_context(tc.tile_pool(name="outpool", bufs=3))
            smpool = ctx.enter_context(tc.tile_pool(name="smpool", bufs=18))
            psq = ctx.enter_context(tc.tile_pool(name="psq", bufs=2, space="PSUM"))
            pso = ctx.enter_context(tc.tile_pool(name="pso", bufs=5, space="PSUM"))
            psw = ctx.enter_context(tc.tile_pool(name="psw", bufs=1, space="PSUM"))

            live = {}

            def a_dma_issue(n, eng=None):
                """x tile DMA issue only."""
                x_t = xpool.tile([128, D_IN], F32, tag="x")
                (eng or nc.sync).dma_start(x_t[:], x_d[n * 128:(n + 1) * 128, :])
                live[("x", n)] = x_t

            def a_stats(n):
                """per-token absmax/scale smalls (DVE)."""
                x_t = live[("x", n)]
                am = smpool.tile([128, 1], F32, tag="am")
                nc.vector.tensor_reduce(
                    am[:], x_t[:], axis=AX, op=op.max, apply_absolute_value=True
                )
                ram = smpool.tile([128, 1], F32, tag="ram")
                nc.vector.reciprocal(ram[:], am[:])
                scl = smpool.tile([128, 1], F32, tag="scl")
                nc.vector.tensor_scalar(scl[:], ram[:], QMAX, None, op0=op.mult)
                live[("am", n)] = am
                live[("scl", n)] = scl

            def a_quant(n):
                """single ACT pass: z16 = fp16(x*scl + 1536) - the fp16 cast IS
                the RNE integer rounding."""
                x_t = live.pop(("x", n))
                scl = live.pop(("scl", n))
                xm = xmpool.tile([128, D_IN], FP16, tag="xm")
                nc.scalar.activation(
                    xm[:], x_t[:], Copy, bias=MAGIC16, scale=scl[:]
                )
                live[("q", n)] = xm

            def a_trans(n, drain="split"):
                """PE fp16 transposes + drain (-1536 -> bf16 ints).
                drain: 'split' = [0:512] ACT + [512:1024] DVE (steady),
                'dve' = whole thing on DVE (head, when ACT is slammed)."""
                q = live.pop(("q", n))
                ps_q = psq.tile([128, D_IN], FP16, tag="ps_q")
                for k in range(KT):
                    nc.tensor.transpose(
                        ps_q[:, k * 128:(k + 1) * 128],
                        q[:, k * 128:(k + 1) * 128],
                        ident[:],
                    )
                qT = qtpool.tile([128, D_IN], BF16, tag="qT")
                if drain == "dve":
                    nc.vector.tensor_scalar(
                        qT[:], ps_q[:], -MAGIC16, None, op0=op.add
                    )
                else:
                    nc.scalar.activation(
                        qT[:, 0:512], ps_q[:, 0:512], Copy, bias=-MAGIC16
                    )
                    nc.vector.tensor_scalar(
                        qT[:, 512:1024], ps_q[:, 512:1024], -MAGIC16, None,
                        op0=op.add,
                    )
                live[("qT", n)] = qT

            def b_coef(n):
                """coef = am*s/127 (DVE small)."""
                am = live.pop(("am", n))
                coef = smpool.tile([128, 1], F32, tag="coef")
                nc.vector.tensor_scalar(coef[:], am[:], scoef[:], None, op0=op.mult)
                live[("coef", n)] = coef

            def b_mm_half(n, oh, qT):
                ps = pso.tile([128, 512], F32, tag="ps")
                for k in range(KT):
                    nc.tensor.matmul(
                        ps[:], qT[:, k * 128:(k + 1) * 128],
                        tT[k][:, oh * 512:(oh + 1) * 512],
                        start=(k == 0), stop=(k == KT - 1),
                    )
                live[("ps", n, oh)] = ps

            def b_scale_act(n, out_sb, lo, hi):
                """out = ps*coef on ACT."""
                oh = 0 if lo < 512 else 1
                ps = live[("ps", n, oh)]
                nc.scalar.activation(
                    out_sb[:, lo:hi], ps[:, lo - oh * 512:hi - oh * 512], Copy,
                    scale=live[("coef", n)][:],
                )

            def b_scale_dve(n, out_sb, lo, hi):
                oh = 0 if lo < 512 else 1
                ps = live[("ps", n, oh)]
                nc.vector.tensor_scalar(
                    out_sb[:, lo:hi], ps[:, lo - oh * 512:hi - oh * 512],
                    live[("coef", n)][:], None, op0=op.mult,
                )

            def b_drop(n):
                live.pop(("ps", n, 0))
                live.pop(("ps", n, 1))
                live.pop(("coef", n))

            def b(n, tail=0):
                """full tile: coef/s_adj, both matmul halves, scales, out DMA.
                tail=1: per-half DMAs.  tail=2: final tile - quarter the oh1
                drain across DVE+ACT with separate DMAs for the shortest
                post-matmul chain."""
                b_coef(n)
                qT = live.pop(("qT", n))
                b_mm_half(n, 0, qT)
                out_sb = outpool.tile([128, D_OUT], BF16, tag="osb")
                b_scale_act(n, out_sb, 0, 512)     # runs while oh1 matmuls go
                if tail:
                    nc.sync.dma_start(
                        o_d[n * 128:(n + 1) * 128, 0:512], out_sb[:, 0:512]
                    )
                b_mm_half(n, 1, qT)
                if tail == 2:
                    b_scale_dve(n, out_sb, 512, 768)
                    nc.sync.dma_start(
                        o_d[n * 128:(n + 1) * 128, 512:768], out_sb[:, 512:768]
                    )
                    b_scale_act(n, out_sb, 768, 1024)
                    nc.scalar.dma_start(
                        o_d[n * 128:(n + 1) * 128, 768:1024], out_sb[:, 768:1024]
                    )
                elif tail == 1:
                    b_scale_dve(n, out_sb, 512, 1024)
                    nc.sync.dma_start(
                        o_d[n * 128:(n + 1) * 128, 512:1024], out_sb[:, 512:1024]
                    )
                else:
                    b_scale_dve(n, out_sb, 512, 1024)
                    nc.sync.dma_start(o_d[n * 128:(n + 1) * 128, :], out_sb[:])
                b_drop(n)

            # ---------------- weight phase + x ramp ------------------------
            with (
                tc.tile_pool(name="wpool", bufs=1) as wpool,
                tc.tile_pool(name="wabs", bufs=2) as wabs_pool,
                tc.tile_pool(name="ypool", bufs=4) as ypool,
            ):
                # Rings balanced at 2.5MB each; x0/x1 in the FIRST slots so
                # tiles 0/1 are fully prepped before s lands; w chunks follow
                # so the |W| mean is ready ~1 chunk-time after the last byte.
                w_t = [wpool.tile([128, D_OUT], F32, name=f"w{k}", tag=f"w{k}") for k in range(KT)]
                a_dma_issue(0, nc.sync)
                a_dma_issue(1, nc.scalar)
                for k in range(2):
                    eng = nc.sync if k % 2 == 0 else nc.scalar
                    eng.dma_start(w_t[k][:], w_d[k * 128:(k + 1) * 128, :])
                # ACT table preload (one load covers all afns): after w1's
                # issue so it delays no transfer (the queue is busy with
                # x1+w1 anyway), but early enough to be done before cs(0).
                nc.scalar.activation(preheat[:], ones_col[:], Abs)
                for k in range(2, KT):
                    eng = nc.sync if k % 2 == 0 else nc.scalar
                    eng.dma_start(w_t[k][:], w_d[k * 128:(k + 1) * 128, :])
                # x2..x5 queue on the sync ring BEHIND the w chunks: hw FIFO
                # per queue is the only real pacing (scheduler floors do not
                # delay hardware - an independent queue pulls immediately and
                # steals HBM from the weight stream, which slips w7 to ~33us).
                # The sync engine stalls on DMA queue-depth while issuing
                # these, but it has nothing else to do in the head.
                for n5 in (2, 3, 4, 5):
                    a_dma_issue(n5, nc.sync)

                # PE p-state warm-up: per-chunk-gated fp32 transposes keep the
                # PE clocked from the first w byte to the first real matmul
                # (idle PE drops to the 0.65/1.2 GHz p-states).
                for k in range(KT):
                    for j in range(2):
                        ps_wu = psw.tile([128, 128], F32, tag="warm")
                        nc.tensor.transpose(
                            ps_wu[:], w_t[k][:, j * 128:(j + 1) * 128],
                            ident_f32[:],
                        )

                # |wT| chunk sums (ACT even / DVE odd) in arrival order; the
                # last chunk is split ACT/DVE so the mean starts ~0.6us after
                # its last byte.  colsum has 9 slots (k7 uses 7 and 8).
                colsum = wpool.tile([128, KT + 1], F32)

                def cs(k):
                    if k % 2 == 0:
                        wabs = wabs_pool.tile([128, D_OUT], F32, tag="wabs")
                        nc.scalar.activation(
                            wabs[:], w_t[k][:], Abs, accum_out=colsum[:, k:k + 1]
                        )
                    else:
                        nc.vector.tensor_reduce(
                            colsum[:, k:k + 1], w_t[k][:], axis=AX, op=op.add,
                            apply_absolute_value=True,
                        )

                # x0/x1 arrive first: full prep (stats+quant+trans+drain) in
                # the head.  Floors track expected data arrival so the
                # scheduler's static per-engine order matches reality (it
                # otherwise e.g. queues scl0 behind cs1, delaying quant0).
                with tc.tile_wait_until(0.0095):
                    a_stats(0)
                    a_quant(0)
                with tc.tile_wait_until(0.010):
                    a_trans(0, drain="dve")
                with tc.tile_wait_until(0.0105):
                    cs(0)
                with tc.tile_wait_until(0.011):
                    a_stats(1)
                    a_quant(1)
                with tc.tile_wait_until(0.0115):
                    cs(1)
                    a_trans(1, drain="dve")
                for k5 in range(2, KT - 1):
                    with tc.tile_wait_until(0.0105 + 0.00125 * k5):
                        cs(k5)
                with tc.tile_wait_until(0.0105 + 0.00125 * 7):
                    wabs7 = wabs_pool.tile([128, 512], F32, tag="wab7")
                    nc.scalar.activation(
                        wabs7[:], w_t[7][:, 0:512], Abs, accum_out=colsum[:, 7:8]
                    )
                    nc.vector.tensor_reduce(
                        colsum[:, 8:9], w_t[7][:, 512:1024], axis=AX, op=op.add,
                        apply_absolute_value=True,
                    )

                colsum2 = wpool.tile([128, 1], F32)
                nc.vector.tensor_reduce(colsum2[:], colsum[:], axis=AX, op=op.add)
                ps_m1 = psw.tile([1, 2], F32, name="ps_m1", tag="warm")
                nc.tensor.matmul(ps_m1[0:1, 0:1], ones_col[:], colsum2[:])
                pair = wpool.tile([1, 2], F32)
                nc.scalar.activation(pair[:, 0:1], ps_m1[0:1, 0:1], Copy, scale=1.0 / (D_OUT * D_IN))
                nc.vector.reciprocal(pair[:, 1:2], pair[:, 0:1])
                ps_m2 = psw.tile([128, 2], F32, name="ps_m2", tag="warm")
                nc.tensor.matmul(ps_m2[:], ones_row[:], pair[:])
                nc.scalar.copy(bc2[:], ps_m2[:])
                nc.vector.tensor_scalar(scoef[:], bc2[:, 0:1], 1.0 / QMAX, None, op0=op.mult)

                qT0 = live.pop(("qT", 0))
                qT1 = live.pop(("qT", 1))

                # ternary-quantize wT in 512-col halves, oh-major, 2 passes:
                # DVE y = w*(1/s) + MAGIC (single-rounding RNE), then ACT
                # t = Sign(y - MAGIC) in {-1,0,1} (Sign IS the clip since
                # |round(w/s)| <= 2).  k in DVE3OP_KS runs the clip as a
                # 3-op DVE chain instead to balance the engines.
                def wq_half(k, oh):
                    sl = slice(oh * 512, (oh + 1) * 512)
                    y = ypool.tile([128, 512], F32, tag="y")
                    reng = nc.gpsimd if k in GPS_ROUND_KS else nc.vector
                    reng.tensor_scalar(
                        y[:], w_t[k][:, sl], bc2[:, 1:2], MAGIC,
                        op0=op.mult, op1=op.add,
                    )
                    if k in DVE3OP_KS or k in GPS3OP_KS:
                        ceng = nc.gpsimd if k in GPS3OP_KS else nc.vector
                        y2 = ypool.tile([128, 512], F32, tag="y2")
                        ceng.tensor_scalar(
                            y2[:], y[:], MAGIC, 1.0, op0=op.subtract, op1=op.min
                        )
                        ceng.tensor_scalar(
                            tT[k][:, sl], y2[:], -1.0, None, op0=op.max
                        )
                    else:
                        nc.scalar.activation(tT[k][:, sl], y[:], Sign, bias=negm[:])

                for wu in range(6):
                    ps_wu = psw.tile([128, 128], BF16, name=f"wu2_{wu}", tag="warm")
                    nc.tensor.transpose(
                        ps_wu[:], qT0[:, wu * 128:(wu + 1) * 128], ident_bf[:]
                    )
                for k in range(KT):
                    wq_half(k, 0)
                with tc.tile_wait_until(0.027):
                    a_stats(2)
                b_coef(0)
                b_coef(1)
                b_mm_half(0, 0, qT0)
                b_mm_half(1, 0, qT1)
                # dep-free fillers: hold the PE p-state through the wait for
                # the oh1 ternary halves (any >0.3us idle drops the clock and
                # the next ~3us of matmuls run at the 1.2GHz p-state).
                for wu in range(20):
                    ps_wu = psw.tile([128, 128], BF16, name=f"wu3_{wu}", tag="warm")
                    nc.tensor.transpose(ps_wu[:], ident_bf[:], ident_bf[:])
                for k in range(KT):
                    wq_half(k, 1)
                b_mm_half(0, 1, qT0)
                with tc.tile_wait_until(0.0275):
                    a_quant(2)        # ACT, after the oh1 rounds (scl2 ready)
                a_trans(2, drain="dve")   # PE slot between the oh1 halves
                b_mm_half(1, 1, qT1)

                with tc.tile_wait_until(0.028):
                    a_stats(3)
                    a_quant(3)    # tile 3's transposes happen at loop n=2

            with tc.tile_wait_until(0.029):
                a_stats(4)
                a_quant(4)
                a_stats(5)

            # Per-iteration scheduler floors: the Tile scheduler list-schedules
            # by its own DMA-latency model and otherwise hoists steady-loop
            # work (absmaxes etc.) ahead of the weight mean-chain, stalling s
            # by ~11us.  Floors pin each iteration near its real cadence.
            for n in range(2, TILES):
                with tc.tile_wait_until(0.0295 + 0.0039 * (n - 2)):
                    if n in (2, 3):
                        # tiles 0/1 output scales ride here: their PSUM banks
                        # are only needed by tile n's second matmul half.
                        n01 = n - 2
                        osb01 = outpool.tile([128, D_OUT], BF16, tag="osb")
                        b_scale_act(n01, osb01, 0, 512)
                        b_scale_dve(n01, osb01, 512, 1024)
                        b_drop(n01)
                        nc.sync.dma_start(
                            o_d[n01 * 128:(n01 + 1) * 128, :], osb01[:]
                        )
                    if n + 4 < TILES:
                        a_dma_issue(n + 4)
                    if n + 3 < TILES:
                        a_quant(n + 3)
                    b(n, tail=max(0, n - (TILES - 3)))
                    if n + 4 < TILES:
                        a_stats(n + 4)
                    if n == 2:
                        a_trans(3)
                    if n + 2 < TILES:
                        a_trans(n + 2)

    _split_multiwaits(nc)
    return nc


_NC_CACHE = None


def _get_nc():
    global _NC_CACHE
    if _NC_CACHE is None:
        _NC_CACHE = build_program()
    return _NC_CACHE


def kernel(x: np.ndarray, weight: np.ndarray, trace: bool = False):
    assert x.shape == (B, S, D_IN) and weight.shape == (D_OUT, D_IN)
    nc = _get_nc()
    xf = np.ascontiguousarray(x.reshape(TOKENS, D_IN), dtype=np.float32)
    wT = np.ascontiguousarray(weight.astype(np.float32, copy=False).T)
    in_maps = [
        {
            "x": xf[c * TOK_PER_CORE:(c + 1) * TOK_PER_CORE],
            "wT": wT,
        }
        for c in range(N_CORES)
    ]
    res = run_bass_kernel_spmd(nc, in_maps, core_ids=list(range(N_CORES)), trace=trace)
    kernel.last_results = res
    out = np.concatenate(
        [np.asarray(res.results[c]["out"]).astype(np.float32) for c in range(N_CORES)],
        axis=0,
    )
    return out.reshape(B, S, D_OUT)


kernel.last_results = None


# revision 19
# speedup vs baseline: 1.1925x; 1.0197x over previous
"""BitLinear (BitNet b1.58) Trainium2 kernel, 8-core data-parallel.

Per core (4096 tokens sharded over batch*seq, weight replicated and fed
host-transposed as wT = W.T, a layout/sharding choice):
    q  = round(x*scale)  integers in [-127,127]   -> exact in fp16/bf16
    t  = clip(round(W/s),-1,1) in {-1,0,1}
    out = (q @ t.T) * (absmax*s/127) per token, stored bf16 (host->fp32).

W-quant is 2 passes: DVE y = w*(1/s) + 12582912 (the fp32 magic add
IS the single-rounding RNE to integer - any smaller bias pre-rounds
the fraction at ulp(bias) and flips boundary weights, each of which
corrupts a whole output column), then ACT t = Sign(y - 12582912):
since |round(w/s)| <= 2, Sign IS the clip to {-1,0,1}.  (A biased
bf16(w/s + 192) variant was tried and rejected: double rounding gave
rel err 2.3e-2.  activation accum_out measures PRE-cast values, so it
cannot supply sum(q) either.)

Engine plan per 128-token tile (steady state, PE-bound ~3.9us):
    SP   : x DMA in, out DMA (both on the sync ring - keeps ACT clean)
    DVE  : absmax reduce, 1/absmax, scl, coef, oh1 output scale,
           [512:1024] drain half
    ACT  : z16 = fp16(x*scl + 1536), [0:512] drain half, oh0 out scale
    PE   : 8 fp16 transposes + 16 bf16 matmuls (fp32 PSUM exact)

Weight phase: rings balanced at 2.5MB each (sync: x0,w0,w2,w4,w6,x2,x4;
scalar: x1,w1,w3,w5,w7,x3,x5) so x0/x1 land first and tiles 0/1 are
fully quantized+transposed BEFORE s is ready; first matmul is then
bound only by the first ternary chunk.  Per-chunk-gated PE warmup
transposes keep the clock at max through the whole stream (PE drops to
0.65/1.2 GHz p-states when idle).  ACT table preloaded with a dummy op
(one load covers all afns).  Last |w| colsum split ACT/DVE.  GPSIMD is
never used for bulk elementwise work (measured 14.7us per 1024-elem
op).  DMA-XBAR transpose rejected: ~1200-descriptor storm per tile.
"""

import numpy as np

import concourse.bass as bass
import concourse.mybir as mybir
from concourse import tile, masks
from concourse.bass_utils import run_bass_kernel_spmd

F32 = mybir.dt.float32
BF16 = mybir.dt.bfloat16
FP16 = mybir.dt.float16

N_CORES = 8
B, S, D_IN, D_OUT = 4, 8192, 1024, 1024
TOKENS = B * S                     # 32768
TOK_PER_CORE = TOKENS // N_CORES   # 4096
TILES = TOK_PER_CORE // 128        # 32
KT = D_IN // 128                   # 8 contraction k-chunks

QMAX = 127.0
MAGIC16 = 1536.0     # z16 = fp16(x*scl + 1536): fp16 ulp=1 on [1024,2048)
MAGIC = 12582912.0   # 1.5 * 2**23: fp32 ulp=1 -> the add rounds RNE
# engine balance for W-quant: chunks whose clip runs as a 3-op DVE
# tensor_scalar chain instead of ACT Sign.  (GpSimd was measured at 7.5us
# per 512-col tensor_scalar with min/max - never offload wq there.)
GPS_ROUND_KS = ()
GPS3OP_KS = ()
DVE3OP_KS = (3,)


def _split_multiwaits(nc):
    """walrus here encodes at most ONE sem wait per instruction; Tile's tail
    drain (and occasionally other insts) carry several.  Split extras into
    single-wait NOPs on the same engine, preserving order."""
    for f in nc.m.functions:
        for bb in f.blocks:
            insts = list(bb.instructions)
            if not any(
                i.sync_info and len(i.sync_info.on_wait) > 1 for i in insts
            ):
                continue
            new = []
            for ins in insts:
                si = ins.sync_info
                if si and len(si.on_wait) > 1:
                    waits = list(si.on_wait)
                    for j, w in enumerate(waits[:-1]):
                        nop = mybir.InstNoOp(
                            name=f"{ins.name}_wsp{j}", ins=[], outs=[]
                        )
                        nop.engine = ins.engine
                        nop.sync_info = mybir.SyncInfo(on_wait=[w], on_update=[])
                        new.append(nop)
                    ins.sync_info = mybir.SyncInfo(
                        on_wait=[waits[-1]], on_update=list(si.on_update)
                    )
                new.append(ins)
            bb.instructions = new


def build_program():
    nc = bass.Bass(trn_type="TRN2")
    x_d = nc.dram_tensor("x", [TOK_PER_CORE, D_IN], F32, kind="ExternalInput")
    w_d = nc.dram_tensor("wT", [D_IN, D_OUT], F32, kind="ExternalInput")
    o_d = nc.dram_tensor("out", [TOK_PER_CORE, D_OUT], BF16, kind="ExternalOutput")

    Copy = mybir.ActivationFunctionType.Copy
    Sign = mybir.ActivationFunctionType.Sign
    Abs = mybir.ActivationFunctionType.Abs
    AX = mybir.AxisListType.X
    op = mybir.AluOpType

    with tile.TileContext(nc) as tc:
        from contextlib import ExitStack

        with ExitStack() as ctx:
            singles = ctx.enter_context(tc.tile_pool(name="singles", bufs=1))

            ident = singles.tile([128, 128], FP16)
            masks.make_identity(nc, ident[:])
            ident_f32 = singles.tile([128, 128], F32)
            masks.make_identity(nc, ident_f32[:])
            ident_bf = singles.tile([128, 128], BF16)
            masks.make_identity(nc, ident_bf[:])
            ones_col = singles.tile([128, 1], F32)
            nc.vector.memset(ones_col[:], 1.0)
            ones_row = singles.tile([1, 128], F32)
            nc.vector.memset(ones_row[:], 1.0)
            bc2 = singles.tile([128, 2], F32)    # [s, 1/s] broadcast to 128 parts
            scoef = singles.tile([128, 1], F32)  # s/127 broadcast
            negm = singles.tile([128, 1], F32)   # -MAGIC bias for the Sign pass
            nc.vector.memset(negm[:], -MAGIC)
            preheat = singles.tile([128, 1], F32)

            tT = [singles.tile([128, D_OUT], BF16, name=f"tT{k}", tag=f"tT{k}") for k in range(KT)]

            xpool = ctx.enter_context(tc.tile_pool(name="xpool", bufs=8))
            xmpool = ctx.enter_context(tc.tile_pool(name="xmpool", bufs=3))
            qtpool = ctx.enter_context(tc.tile_pool(name="qtpool", bufs=6))
            outpool = ctx.enter_context(tc.tile_pool(name="outpool", bufs=3))
            smpool = ctx.enter_context(tc.tile_pool(name="smpool", bufs=18))
            psq = ctx.enter_context(tc.tile_pool(name="psq", bufs=2, space="PSUM"))
            pso = ctx.enter_context(tc.tile_pool(name="pso", bufs=5, space="PSUM"))
            psw = ctx.enter_context(tc.tile_pool(name="psw", bufs=1, space="PSUM"))

            live = {}

            def a_dma_issue(n, eng=None):
                """x tile DMA issue only."""
                x_t = xpool.tile([128, D_IN], F32, tag="x")
                (eng or nc.sync).dma_start(x_t[:], x_d[n * 128:(n + 1) * 128, :])
                live[("x", n)] = x_t

            def a_stats(n):
                """per-token absmax/scale smalls (DVE)."""
                x_t = live[("x", n)]
                am = smpool.tile([128, 1], F32, tag="am")
                nc.vector.tensor_reduce(
                    am[:], x_t[:], axis=AX, op=op.max, apply_absolute_value=True
                )
                ram = smpool.tile([128, 1], F32, tag="ram")
                nc.vector.reciprocal(ram[:], am[:])
                scl = smpool.tile([128, 1], F32, tag="scl")
                nc.vector.tensor_scalar(scl[:], ram[:], QMAX, None, op0=op.mult)
                live[("am", n)] = am
                live[("scl", n)] = scl

            def a_quant(n):
                """single ACT pass: z16 = fp16(x*scl + 1536) - the fp16 cast IS
                the RNE integer rounding."""
                x_t = live.pop(("x", n))
                scl = live.pop(("scl", n))
                xm = xmpool.tile([128, D_IN], FP16, tag="xm")
                nc.scalar.activation(
                    xm[:], x_t[:], Copy, bias=MAGIC16, scale=scl[:]
                )
                live[("q", n)] = xm

            def a_trans(n, drain="split"):
                """PE fp16 transposes + drain (-1536 -> bf16 ints).
                drain: 'split' = [0:512] ACT + [512:1024] DVE (steady),
                'dve' = whole thing on DVE (head, when ACT is slammed)."""
                q = live.pop(("q", n))
                ps_q = psq.tile([128, D_IN], FP16, tag="ps_q")
                for k in range(KT):
                    nc.tensor.transpose(
                        ps_q[:, k * 128:(k + 1) * 128],
                        q[:, k * 128:(k + 1) * 128],
                        ident[:],
                    )
                qT = qtpool.tile([128, D_IN], BF16, tag="qT")
                if drain == "dve":
                    nc.vector.tensor_scalar(
                        qT[:], ps_q[:], -MAGIC16, None, op0=op.add
                    )
                else:
                    nc.scalar.activation(
                        qT[:, 0:512], ps_q[:, 0:512], Copy, bias=-MAGIC16
                    )
                    nc.vector.tensor_scalar(
                        qT[:, 512:1024], ps_q[:, 512:1024], -MAGIC16, None,
                        op0=op.add,
                    )
                live[("qT", n)] = qT

            def b_coef(n):
                """coef = am*s/127 (DVE small)."""
                am = live.pop(("am", n))
                coef = smpool.tile([128, 1], F32, tag="coef")
                nc.vector.tensor_scalar(coef[:], am[:], scoef[:], None, op0=op.mult)
                live[("coef", n)] = coef

            def b_mm_half(n, oh, qT):
                ps = pso.tile([128, 512], F32, tag="ps")
                for k in range(KT):
                    nc.tensor.matmul(
                        ps[:], qT[:, k * 128:(k + 1) * 128],
                        tT[k][:, oh * 512:(oh + 1) * 512],
                        start=(k == 0), stop=(k == KT - 1),
                    )
                live[("ps", n, oh)] = ps

            def b_scale_act(n, out_sb, lo, hi):
                """out = ps*coef on ACT."""
                oh = 0 if lo < 512 else 1
                ps = live[("ps", n, oh)]
                nc.scalar.activation(
                    out_sb[:, lo:hi], ps[:, lo - oh * 512:hi - oh * 512], Copy,
                    scale=live[("coef", n)][:],
                )

            def b_scale_dve(n, out_sb, lo, hi):
                oh = 0 if lo < 512 else 1
                ps = live[("ps", n, oh)]
                nc.vector.tensor_scalar(
                    out_sb[:, lo:hi], ps[:, lo - oh * 512:hi - oh * 512],
                    live[("coef", n)][:], None, op0=op.mult,
                )

            def b_drop(n):
                live.pop(("ps", n, 0))
                live.pop(("ps", n, 1))
                live.pop(("coef", n))

            def b(n, tail=0):
                """full tile: coef/s_adj, both matmul halves, scales, out DMA.
                tail=1: per-half DMAs.  tail=2: final tile - quarter the oh1
                drain across DVE+ACT with separate DMAs for the shortest
                post-matmul chain."""
                b_coef(n)
                qT = live.pop(("qT", n))
                b_mm_half(n, 0, qT)
                out_sb = outpool.tile([128, D_OUT], BF16, tag="osb")
                b_scale_act(n, out_sb, 0, 512)     # runs while oh1 matmuls go
                if tail:
                    nc.sync.dma_start(
                        o_d[n * 128:(n + 1) * 128, 0:512], out_sb[:, 0:512]
                    )
                b_mm_half(n, 1, qT)
                if tail == 2:
                    b_scale_dve(n, out_sb, 512, 768)
                    nc.sync.dma_start(
                        o_d[n * 128:(n + 1) * 128, 512:768], out_sb[:, 512:768]
                    )
                    b_scale_act(n, out_sb, 768, 1024)
                    nc.scalar.dma_start(
                        o_d[n * 128:(n + 1) * 128, 768:1024], out_sb[:, 768:1024]
                    )
                elif tail == 1:
                    b_scale_dve(n, out_sb, 512, 1024)
                    nc.sync.dma_start(
                        o_d[n * 128:(n + 1) * 128, 512:1024], out_sb[:, 512:1024]
                    )
                else:
                    b_scale_dve(n, out_sb, 512, 1024)
                    nc.sync.dma_start(o_d[n * 128:(n + 1) * 128, :], out_sb[:])
                b_drop(n)

            # ---------------- weight phase + x ramp ------------------------
            with (
                tc.tile_pool(name="wpool", bufs=1) as wpool,
                tc.tile_pool(name="wabs", bufs=2) as wabs_pool,
                tc.tile_pool(name="ypool", bufs=4) as ypool,
            ):
                # Rings balanced at 2.5MB each; x0/x1 in the FIRST slots so
                # tiles 0/1 are fully prepped before s lands; w chunks follow
                # so the |W| mean is ready ~1 chunk-time after the last byte.
                w_t = [wpool.tile([128, D_OUT], F32, name=f"w{k}", tag=f"w{k}") for k in range(KT)]
                for k in range(2):
                    eng = nc.sync if k % 2 == 0 else nc.scalar
                    eng.dma_start(w_t[k][:], w_d[k * 128:(k + 1) * 128, :])
                # ACT table preload (one load covers all afns): after w1's
                # issue so it delays no transfer, early enough to precede cs0.
                nc.scalar.activation(preheat[:], ones_col[:], Abs)
                a_dma_issue(0, nc.sync)
                a_dma_issue(1, nc.scalar)
                for k in range(2, KT):
                    eng = nc.sync if k % 2 == 0 else nc.scalar
                    eng.dma_start(w_t[k][:], w_d[k * 128:(k + 1) * 128, :])
                # x2..x5 queue BEHIND the w chunks on both rings: hw FIFO per
                # queue is the only real pacing (an independent queue pulls
                # immediately and steals HBM from the weight stream).  The
                # issuing engines stall on DMA queue-depth, which is harmless
                # on sync and accounted for on scalar (issues precede the
                # colsums in its program order).
                a_dma_issue(2, nc.sync)
                a_dma_issue(3, nc.scalar)
                a_dma_issue(4, nc.sync)
                a_dma_issue(5, nc.scalar)

                # PE p-state warm-up: per-chunk-gated fp32 transposes keep the
                # PE clocked from the first w byte to the first real matmul
                # (idle PE drops to the 0.65/1.2 GHz p-states).
                for k in range(KT):
                    with tc.tile_wait_until(0.0115 + 0.0013 * k):
                        for j in range(2):
                            ps_wu = psw.tile([128, 128], F32, tag="warm")
                            nc.tensor.transpose(
                                ps_wu[:], w_t[k][:, j * 128:(j + 1) * 128],
                                ident_f32[:],
                            )

                # |wT| chunk sums (ACT even / DVE odd) in arrival order; the
                # last chunk is split ACT/DVE so the mean starts ~0.6us after
                # its last byte.  colsum has 9 slots (k7 uses 7 and 8).
                colsum = wpool.tile([128, KT + 1], F32)

                def cs(k):
                    if k % 2 == 0:
                        wabs = wabs_pool.tile([128, D_OUT], F32, tag="wabs")
                        nc.scalar.activation(
                            wabs[:], w_t[k][:], Abs, accum_out=colsum[:, k:k + 1]
                        )
                    else:
                        nc.vector.tensor_reduce(
                            colsum[:, k:k + 1], w_t[k][:], axis=AX, op=op.add,
                            apply_absolute_value=True,
                        )

                # x0/x1 arrive first: full prep (stats+quant+trans+drain) in
                # the head.  Floors track expected data arrival so the
                # scheduler's static per-engine order matches reality (it
                # otherwise e.g. queues scl0 behind cs1, delaying quant0).
                with tc.tile_wait_until(0.0117):
                    cs(0)
                with tc.tile_wait_until(0.014):
                    a_stats(0)
                with tc.tile_wait_until(0.0155):
                    a_quant(0)
                with tc.tile_wait_until(0.017):
                    a_trans(0, drain="dve")
                with tc.tile_wait_until(0.0130):
                    cs(1)
                with tc.tile_wait_until(0.0165):
                    a_stats(1)
                with tc.tile_wait_until(0.018):
                    a_quant(1)
                with tc.tile_wait_until(0.0195):
                    a_trans(1, drain="dve")
                for k5 in range(2, KT - 1):
                    with tc.tile_wait_until(0.0115 + 0.0013 * k5):
                        cs(k5)
                with tc.tile_wait_until(0.0115 + 0.0013 * 7):
                    wabs7 = wabs_pool.tile([128, 512], F32, tag="wab7")
                    nc.scalar.activation(
                        wabs7[:], w_t[7][:, 0:512], Abs, accum_out=colsum[:, 7:8]
                    )
                    nc.vector.tensor_reduce(
                        colsum[:, 8:9], w_t[7][:, 512:1024], axis=AX, op=op.add,
                        apply_absolute_value=True,
                    )

                colsum2 = wpool.tile([128, 1], F32)
                nc.vector.tensor_reduce(colsum2[:], colsum[:], axis=AX, op=op.add)
                ps_m1 = psw.tile([1, 2], F32, name="ps_m1", tag="warm")
                nc.tensor.matmul(ps_m1[0:1, 0:1], ones_col[:], colsum2[:])
                pair = wpool.tile([1, 2], F32)
                nc.scalar.activation(pair[:, 0:1], ps_m1[0:1, 0:1], Copy, scale=1.0 / (D_OUT * D_IN))
                nc.vector.reciprocal(pair[:, 1:2], pair[:, 0:1])
                ps_m2 = psw.tile([128, 2], F32, name="ps_m2", tag="warm")
                nc.tensor.matmul(ps_m2[:], ones_row[:], pair[:])
                nc.scalar.copy(bc2[:], ps_m2[:])
                nc.vector.tensor_scalar(scoef[:], bc2[:, 0:1], 1.0 / QMAX, None, op0=op.mult)

                qT0 = live.pop(("qT", 0))
                qT1 = live.pop(("qT", 1))

                # ternary-quantize wT in 512-col halves, oh-major, 2 passes:
                # DVE y = w*(1/s) + MAGIC (single-rounding RNE), then ACT
                # t = Sign(y - MAGIC) in {-1,0,1} (Sign IS the clip since
                # |round(w/s)| <= 2).  k in DVE3OP_KS runs the clip as a
                # 3-op DVE chain instead to balance the engines.
                def wq_half(k, oh):
                    sl = slice(oh * 512, (oh + 1) * 512)
                    y = ypool.tile([128, 512], F32, tag="y")
                    reng = nc.gpsimd if k in GPS_ROUND_KS else nc.vector
                    reng.tensor_scalar(
                        y[:], w_t[k][:, sl], bc2[:, 1:2], MAGIC,
                        op0=op.mult, op1=op.add,
                    )
                    if k in DVE3OP_KS or k in GPS3OP_KS:
                        ceng = nc.gpsimd if k in GPS3OP_KS else nc.vector
                        y2 = ypool.tile([128, 512], F32, tag="y2")
                        ceng.tensor_scalar(
                            y2[:], y[:], MAGIC, 1.0, op0=op.subtract, op1=op.min
                        )
                        ceng.tensor_scalar(
                            tT[k][:, sl], y2[:], -1.0, None, op0=op.max
                        )
                    else:
                        nc.scalar.activation(tT[k][:, sl], y[:], Sign, bias=negm[:])

                with tc.tile_wait_until(0.0198):
                    for wu in range(6):
                        ps_wu = psw.tile([128, 128], BF16, name=f"wu2_{wu}", tag="warm")
                        nc.tensor.transpose(
                            ps_wu[:], qT0[:, wu * 128:(wu + 1) * 128], ident_bf[:]
                        )
                for k in range(KT):
                    wq_half(k, 0)
                with tc.tile_wait_until(0.0238):
                    a_stats(2)
                b_coef(0)
                b_coef(1)
                b_mm_half(0, 0, qT0)
                b_mm_half(1, 0, qT1)
                # dep-free fillers: hold the PE p-state through the wait for
                # the oh1 ternary halves (any >0.3us idle drops the clock and
                # the next ~3us of matmuls run at the 1.2GHz p-state).
                with tc.tile_wait_until(0.021):
                    for wu in range(20):
                        ps_wu = psw.tile([128, 128], BF16, name=f"wu3_{wu}", tag="warm")
                        nc.tensor.transpose(ps_wu[:], ident_bf[:], ident_bf[:])
                for k in range(KT):
                    wq_half(k, 1)
                b_mm_half(0, 1, qT0)
                with tc.tile_wait_until(0.0252):
                    a_quant(2)        # ACT, after the oh1 rounds (scl2 ready)
                a_trans(2, drain="dve")   # PE slot between the oh1 halves
                b_mm_half(1, 1, qT1)

                with tc.tile_wait_until(0.0258):
                    a_stats(3)
                    a_quant(3)    # tile 3's transposes happen at loop n=2

            with tc.tile_wait_until(0.027):
                a_stats(4)
                a_quant(4)
                a_stats(5)

            # Per-iteration scheduler floors: the Tile scheduler list-schedules
            # by its own DMA-latency model and otherwise hoists steady-loop
            # work (absmaxes etc.) ahead of the weight mean-chain, stalling s
            # by ~11us.  Floors pin each iteration near its real cadence.
            for n in range(2, TILES):
                with tc.tile_wait_until(0.028 + 0.0039 * (n - 2)):
                    if n in (2, 3):
                        # tiles 0/1 output scales ride here: their PSUM banks
                        # are only needed by tile n's second matmul half.
                        n01 = n - 2
                        osb01 = outpool.tile([128, D_OUT], BF16, tag="osb")
                        b_scale_act(n01, osb01, 0, 512)
                        b_scale_dve(n01, osb01, 512, 1024)
                        b_drop(n01)
                        nc.sync.dma_start(
                            o_d[n01 * 128:(n01 + 1) * 128, :], osb01[:]
                        )
                    if n + 4 < TILES:
                        a_dma_issue(n + 4)
                    if n + 3 < TILES:
                        a_quant(n + 3)
                    b(n, tail=max(0, n - (TILES - 3)))
                    if n + 4 < TILES:
                        a_stats(n + 4)
                    if n == 2:
                        a_trans(3)
                    if n + 2 < TILES:
                        a_trans(n + 2)

    _split_multiwaits(nc)
    return nc


_NC_CACHE = None


def _get_nc():
    global _NC_CACHE
    if _NC_CACHE is None:
        _NC_CACHE = build_program()
    return _NC_CACHE


def kernel(x: np.ndarray, weight: np.ndarray, trace: bool = False):
    assert x.shape == (B, S, D_IN) and weight.shape == (D_OUT, D_IN)
    nc = _get_nc()
    xf = np.ascontiguousarray(x.reshape(TOKENS, D_IN), dtype=np.float32)
    wT = np.ascontiguousarray(weight.astype(np.float32, copy=False).T)
    in_maps = [
        {
            "x": xf[c * TOK_PER_CORE:(c + 1) * TOK_PER_CORE],
            "wT": wT,
        }
        for c in range(N_CORES)
    ]
    res = run_bass_kernel_spmd(nc, in_maps, core_ids=list(range(N_CORES)), trace=trace)
    kernel.last_results = res
    out = np.concatenate(
        [np.asarray(res.results[c]["out"]).astype(np.float32) for c in range(N_CORES)],
        axis=0,
    )
    return out.reshape(B, S, D_OUT)


kernel.last_results = None
